# revision 20
# baseline (speedup 1.0000x reference)
"""MoE (top-1, capacity_factor=1) Trainium2 Bass kernel, expert-parallel over
8 NeuronCores. Self-contained: imports only numpy + concourse (/opt).

Per-core k (expert k resident):
  gate GEMM (fp32 exact) on its 1/8 token shard -> AllGather logits ->
  replicated routing (one-hot, global cumsum via triangular matmuls,
  capacity mask) -> slot->token table via gpsimd sparse_gather ->
  indirect row-gather of x (fp16) + PE transposes -> dispT [m, c] ->
  GEMM1 (fp16, w1 resident in SBUF) -> hT [h, c] with fused ReLU+b1 ->
  GEMM2 (fp16, w2 streamed) -> eoT [m, c], + b2 + gate scaling -> EOT (f16).
Host: scatter rows eo = EOT.T back by token id (A table), empties G==0.

v2: PE kept warm from t=0; dummy collective to absorb CC-stack warmup;
SBUF->SBUF relayouts instead of DRAM round-trips where legal; strided
loads split across sync/scalar queues; x gathered as fp16 (half bytes);
w1 fully resident; EOT in fp16.
"""
import sys

sys.path.insert(0, '/opt/trn_rl_repo')

import numpy as np
import concourse.bass as bass
import concourse.tile as tile
import concourse.mybir as mybir
from concourse import bacc
from concourse.bass_utils import run_bass_kernel_spmd
from concourse.masks import make_upper_triangular, make_identity

N_CORES = 8
B, SQ, M, E, H = 4, 2048, 1024, 8, 4096
S = B * SQ            # 8192 tokens
C = S // E            # 1024 capacity
NT = S // 128         # 64 token tiles
NTS = NT // N_CORES   # 8 token tiles per core shard
CBLK = 512            # c-block for GEMM1/GEMM2
NCB = C // CBLK       # 2 c-blocks
F_IN = (S + C) // 16  # 576  sparse_gather input free size
F_OUT = C // 16       # 64

f32 = mybir.dt.float32
f16 = mybir.dt.float16
bf16 = mybir.dt.bfloat16
i32 = mybir.dt.int32
u32 = mybir.dt.uint32
AX = mybir.AxisListType
OP = mybir.AluOpType
ACTF = mybir.ActivationFunctionType


def _split_multi_waits(nc):
    """This walrus build accepts at most ONE sync-wait per instruction.
    Split extras into same-engine NOPs inserted just before."""
    from concourse.mybir import SyncInfo
    n = 0
    for bb in list(nc.main_func.blocks):
        insts = bb.instructions  # live shared list
        for ins in list(insts):
            si = ins.sync_info
            if si is None or len(si.on_wait) <= 1:
                continue
            waits = list(si.on_wait)
            idx = insts.index(ins)
            for j, w in enumerate(waits[:-1]):
                nop = nc.engines[ins.engine].nop(nofuse=True, hint="waitsplit")
                ni = nop.ins
                cur = nc.cur_bb.bb.instructions
                if ni in cur:
                    cur.remove(ni)
                ni.sync_info = SyncInfo(on_wait=[w], on_update=[])
                insts.insert(idx + j, ni)
                n += 1
            ins.sync_info = SyncInfo(on_wait=[waits[-1]], on_update=si.on_update)
    return n


def r3(ap, e=E):
    return ap.rearrange("p (t e) -> p t e", e=e)


def build():
    nc = bacc.Bacc("TRN2", target_bir_lowering=False, debug=False,
                   num_devices=N_CORES)

    x16_ap = nc.dram_tensor("x16", [S, M], f16, kind="ExternalInput").ap()
    xts_ap = nc.dram_tensor("xts", [M, S // N_CORES], f32, kind="ExternalInput").ap()
    wg_ap = nc.dram_tensor("wg", [M, E], f32, kind="ExternalInput").ap()
    w1_ap = nc.dram_tensor("w1", [M, H], f16, kind="ExternalInput").ap()
    b1c_ap = nc.dram_tensor("b1c", [128, H // 128], f32, kind="ExternalInput").ap()
    w2t_ap = nc.dram_tensor("w2t", [M // 128, 128, H], f16, kind="ExternalInput").ap()
    b2c_ap = nc.dram_tensor("b2c", [128, M // 128], f32, kind="ExternalInput").ap()
    eid_ap = nc.dram_tensor("eid", [128, 1], f32, kind="ExternalInput").ap()

    eot_ap = nc.dram_tensor("EOT", [M, C], f16, kind="ExternalOutput").ap()
    a_ap = nc.dram_tensor("A", [C, 1], i32, kind="ExternalOutput").ap()
    g_ap = nc.dram_tensor("G", [C, 1], f32, kind="ExternalOutput").ap()

    with tile.TileContext(nc) as tc:
        with tc.tile_pool(name="consts", bufs=1) as cp, \
             tc.tile_pool(name="rt", bufs=1) as rt, \
             tc.tile_pool(name="dram", bufs=1, space="DRAM") as dram:

            # gate inputs first: these DMAs gate the whole pipeline.
            # xts strips split across both HWDGE trigger queues.
            wg_sb = cp.tile([128, (M // 128) * E], f32)   # [p, (mc, e)]
            nc.sync.dma_start(wg_sb[:].rearrange("p (t e) -> p t e", e=E),
                              wg_ap.rearrange("(mc p) e -> p mc e", p=128))
            xts_pool_cm = tc.tile_pool(name="xts", bufs=1)
            xp = xts_pool_cm.__enter__()
            xts_strips = []
            for mc in range(M // 128):
                st = xp.tile([128, S // N_CORES], f32, tag=f"xts{mc}",
                             name=f"xts{mc}")
                eng = nc.sync if mc % 2 == 0 else nc.scalar
                eng.dma_start(st[:], xts_ap[mc * 128:(mc + 1) * 128, :])
                xts_strips.append(st)

            # w1 fully resident in SBUF. The DMAs are gated behind the gate
            # GEMM output (WAW on w1_sb via the seed copy below) so the 8 MiB
            # stream does not compete with the latency-critical xts load.
            w1_sb = cp.tile([128, (M // 128) * H], f16)   # [p, (mc, h)]

            # ---------------- constants ----------------
            TRI = cp.tile([128, 128], f32)       # TRI[q,p]=1 iff q<=p
            make_upper_triangular(nc, TRI[:], val=1.0, diag=True)
            TRIS = cp.tile([64, 64], f32)        # strict upper
            make_upper_triangular(nc, TRIS[:], val=1.0, diag=False)
            IDN = cp.tile([128, 128], f32)
            make_identity(nc, IDN[:])
            IDN16 = cp.tile([128, 128], f16)
            make_identity(nc, IDN16[:])
            ones_col = cp.tile([128, 1], f32)
            nc.vector.memset(ones_col[:], 1.0)
            ones_row = cp.tile([1, 128], f32)
            nc.vector.memset(ones_row[:], 1.0)
            iota_e = cp.tile([128, NT * E], f32)
            nc.gpsimd.iota(r3(iota_e[:]), pattern=[[0, NT], [1, E]], base=0,
                           channel_multiplier=0,
                           allow_small_or_imprecise_dtypes=True)
            tokf = cp.tile([128, NT], f32)
            nc.gpsimd.iota(tokf[:], pattern=[[128, NT]], base=0,
                           channel_multiplier=1,
                           allow_small_or_imprecise_dtypes=True)
            eid_sb = cp.tile([128, 1], f32)
            nc.scalar.dma_start(eid_sb[:], eid_ap[:])
            b1c = cp.tile([128, H // 128], f32)
            nc.scalar.dma_start(b1c[:], b1c_ap[:])
            b2c = cp.tile([128, M // 128], f32)
            nc.scalar.dma_start(b2c[:], b2c_ap[:])

            # preload the sparse_gather ucode library early
            dumv = cp.tile([16, 8], f32)
            nc.vector.memset(dumv[:], 1.0)
            dumo = cp.tile([16, 8], f32)
            dumn = cp.tile([1, 1], u32)
            nc.gpsimd.sparse_gather(dumo[:], dumv[:], num_found=dumn[:])
            zoff = cp.tile([128, 1], i32)
            nc.vector.memset(zoff[:], 0)
            dumg = cp.tile([128, 64], f16)
            nc.gpsimd.indirect_dma_start(
                out=dumg[:], out_offset=None, in_=x16_ap[:, 0:64],
                in_offset=bass.IndirectOffsetOnAxis(ap=zoff[:], axis=0))

            # PE warm machinery: keep the HAM activity monitor fed so real
            # matmuls run at full clock. PSUM banks are pool-granular, so the
            # warm pool opens/closes around phases that need all 8 banks.
            warm_in = cp.tile([128, 64], bf16)
            nc.vector.memset(warm_in[:], 1.0)
            warm_w = cp.tile([128, 64], bf16)
            nc.vector.memset(warm_w[:], 1.0)
            warm_state = {}

            def warm_open():
                cm = tc.tile_pool(name="warm_ps", bufs=1, space="PSUM")
                pool = cm.__enter__()
                warm_state['cm'] = cm
                warm_state['out'] = pool.tile([64, 64], f32, tag="warm",
                                              name="warmtile")

            def warm_close():
                warm_state.pop('cm').__exit__(None, None, None)

            def pe_warm(n):
                for _ in range(n):
                    nc.tensor.matmul(warm_state['out'][:], warm_w[:],
                                     warm_in[:], start=True, stop=True,
                                     skip_group_check=True)

            warm_in32 = cp.tile([128, 64], f32)
            nc.vector.memset(warm_in32[:], 1.0)

            def warm_anchor(src_ap, np_):
                # a matmul that READS src_ap chains the warm burst behind
                # that tile's producer (the scheduler reorders freely, so
                # data deps are the only reliable sequencing tool).
                nc.tensor.matmul(warm_state['out'][0:src_ap.shape[1], 0:64],
                                 src_ap, warm_in32[:, 0:64],
                                 start=True, stop=True, skip_group_check=True)
                pe_warm(np_)

            # ~4us of initial PE busy to trip the HAM SHORT window before
            # the gate GEMM begins, then a wg-anchored burst to keep it warm
            # until the first xts strip lands.
            warm_open()
            pe_warm(56)
            warm_anchor(wg_sb[:, 0:8], 40)
            warm_close()

            # ---------------- phase 1: gate GEMM (fp32) ----------------
            Lg_sb = rt.tile([128, NTS * E], f32)          # own shard logits
            Lg_in = dram.tile([S // N_CORES, E], f32)
            Lg_all = dram.tile([S, E], f32, addr_space="Shared")

            with tc.tile_pool(name="psg_p", bufs=1, space="PSUM") as psg_p:
                psgs = [psg_p.tile([128, E], f32, tag=f"psg{t}", name=f"psg{t}")
                        for t in range(NTS)]
                for mc in range(M // 128):
                    st = xts_strips[mc]
                    for t in range(NTS):
                        nc.tensor.matmul(
                            psgs[t][:], st[:, t * 128:(t + 1) * 128],
                            wg_sb[:, mc * E:(mc + 1) * E],
                            start=(mc == 0), stop=(mc == M // 128 - 1))
                for t in range(NTS):
                    nc.vector.tensor_copy(Lg_sb[:, t * E:(t + 1) * E], psgs[t][:])
            xts_pool_cm.__exit__(None, None, None)

            nc.sync.dma_start(Lg_in[:].rearrange("(t p) e -> p t e", p=128),
                              r3(Lg_sb[:]))
            nc.gpsimd.collective_compute(
                "AllGather", OP.bypass,
                replica_groups=[list(range(N_CORES))],
                ins=[Lg_in[:]], outs=[Lg_all[:]])
            warm_open()
            pe_warm(16)

            # release the w1 stream now that the gate GEMM's inputs are done:
            # seed WAW dep on w1_sb from Lg_sb, then the 4 big loads.
            nc.vector.tensor_copy(w1_sb[0:1, 0:1], Lg_sb[0:1, 0:1])
            for qq in range(4):
                nc.scalar.dma_start(
                    w1_sb[:].rearrange("p (mc h) -> p mc h", h=H)
                    [:, qq * 2:(qq + 1) * 2, :],
                    w1_ap.rearrange("(mc p) h -> p mc h", p=128)
                    [:, qq * 2:(qq + 1) * 2, :])

            # dummy indirect gather gated on Lg_sb: keeps the dynamic DMA
            # queue spun up so the real gathers don't pay cold-start.
            zoff2 = cp.tile([128, 1], i32)
            nc.vector.tensor_scalar(out=zoff2[:], in0=Lg_sb[:, 0:1],
                                    scalar1=0.0, scalar2=None, op0=OP.mult)
            dumg2 = cp.tile([128, 64], f16)
            nc.gpsimd.indirect_dma_start(
                out=dumg2[:], out_offset=None, in_=x16_ap[:, 0:64],
                in_offset=bass.IndirectOffsetOnAxis(ap=zoff2[:], axis=0))

            bigp_cm = tc.tile_pool(name="big", bufs=1)
            bigp = bigp_cm.__enter__()
            wk_cm = tc.tile_pool(name="wk", bufs=2)
            wk = wk_cm.__enter__()

            # ---------------- phase 2: routing (replicated) ----------------
            L_all = rt.tile([128, NT * E], f32)
            for lc in range(8):
                tchunk = NT // 8
                eng = nc.sync if lc % 2 == 0 else nc.scalar
                eng.dma_start(
                    r3(L_all[:, lc * tchunk * E:(lc + 1) * tchunk * E]),
                    Lg_all[lc * tchunk * 128:(lc + 1) * tchunk * 128, :]
                    .rearrange("(t p) e -> p t e", p=128))
            L3 = r3(L_all[:])

            lmax = rt.tile([128, NT], f32)
            nc.vector.tensor_reduce(lmax[:], L3, axis=AX.X, op=OP.max)
            lmax_b = lmax[:].rearrange("p (t o) -> p t o", o=1) \
                            .to_broadcast([128, NT, E])
            dd = rt.tile([128, NT * E], f32)
            nc.vector.tensor_tensor(out=r3(dd[:]), in0=L3, in1=lmax_b,
                                    op=OP.subtract)
            expd = rt.tile([128, NT * E], f32)
            nc.scalar.activation(expd[:], dd[:], ACTF.Exp)
            ssum = rt.tile([128, NT], f32)
            nc.vector.tensor_reduce(ssum[:], r3(expd[:]), axis=AX.X, op=OP.add)
            gv = rt.tile([128, NT], f32)
            nc.vector.reciprocal(gv[:], ssum[:])

            oh = rt.tile([128, NT * E], f32)
            nc.vector.tensor_tensor(out=r3(oh[:]), in0=L3, in1=lmax_b,
                                    op=OP.is_equal)
            tmp = rt.tile([128, NT * E], f32)
            nc.vector.tensor_mul(tmp[:], oh[:], iota_e[:])
            eidx = rt.tile([128, NT], f32)
            nc.vector.tensor_reduce(eidx[:], r3(tmp[:]), axis=AX.X, op=OP.add)

            ps_rt_cm = tc.tile_pool(name="ps_rt", bufs=1, space="PSUM")
            ps_rt = ps_rt_cm.__enter__()
            pos_ps = ps_rt.tile([128, NT * E], f32, tag="pos")
            nc.tensor.matmul(pos_ps[:], TRI[:], oh[:], start=True, stop=False)
            cnt_ps = ps_rt.tile([64, 8], f32, tag="cnt")
            oh3 = oh[:].rearrange("p (t e) -> p e t", e=E)
            for e in range(E):
                nc.tensor.matmul(cnt_ps[:, e:e + 1], oh3[:, e, :], ones_col[:],
                                 start=True, stop=True)
            cnt64 = rt.tile([64, 8], f32)
            nc.vector.tensor_copy(cnt64[:], cnt_ps[:])
            offs_ps = ps_rt.tile([64, 8], f32, tag="offs")
            nc.tensor.matmul(offs_ps[:], TRIS[:], cnt64[:], start=True, stop=True)
            offs_sb = rt.tile([64, 8], f32)
            nc.vector.tensor_copy(offs_sb[:], offs_ps[:])
            # direct SBUF->SBUF relayout [64p,8f] -> [1p,512f] (row-major both)
            offs_flat = rt.tile([1, NT * E], f32)
            nc.scalar.dma_start(offs_flat[0:1, :], offs_sb[:])
            nc.tensor.matmul(pos_ps[:], ones_row[:], offs_flat[:],
                             start=False, stop=True)

            pe_warm(8)
            pm = rt.tile([128, NT * E], f32)
            nc.vector.tensor_mul(pm[:], pos_ps[:], oh[:])
            pos_tok = rt.tile([128, NT], f32)
            nc.vector.tensor_reduce(pos_tok[:], r3(pm[:]), axis=AX.X, op=OP.add)
            nc.vector.tensor_scalar_add(pos_tok[:], pos_tok[:], -1.0)

            keep = rt.tile([128, NT], f32)
            nc.vector.tensor_scalar(out=keep[:], in0=pos_tok[:],
                                    scalar1=float(C), scalar2=None, op0=OP.is_lt)
            mine = rt.tile([128, NT], f32)
            nc.vector.tensor_scalar(out=mine[:], in0=eidx[:],
                                    scalar1=eid_sb[:, 0:1], scalar2=None,
                                    op0=OP.is_equal)
            sel = rt.tile([128, NT], f32)
            nc.vector.tensor_mul(sel[:], mine[:], keep[:])
            gvk = rt.tile([128, NT], f32)
            nc.vector.tensor_mul(gvk[:], gv[:], keep[:])

            # packed payload: tokid*2048 + round(gv*2047) if sel else -1;
            # cols NT..NT+7 stay 0 (always-kept sentinels -> A=0, G=0)
            XCOL = NT + C // 128
            mtok = rt.tile([128, XCOL], f32)
            nc.vector.memset(mtok[:, NT:], 0.0)
            nc.vector.tensor_scalar(out=mtok[:, :NT], in0=tokf[:], scalar1=2048.0,
                                    scalar2=1.0, op0=OP.mult, op1=OP.add)
            gq = rt.tile([128, NT], f32)
            nc.vector.tensor_scalar_mul(gq[:], gvk[:], 2047.0)
            nc.vector.tensor_add(mtok[:, :NT], mtok[:, :NT], gq[:])
            nc.vector.tensor_mul(mtok[:, :NT], mtok[:, :NT], sel[:])
            nc.vector.tensor_scalar_add(mtok[:, :NT], mtok[:, :NT], -1.0)

            # ------ sparse_gather compaction: slot -> (tokid, gate) ------
            # DRAM staging buffers are PERMUTED (partition-major for the
            # [16,F] sparse_gather view) so each hop moves >=32B runs
            # instead of 4B-strided packets.
            Vd1q = dram.tile([S + C], f32)     # layout: q*576 + t*8 + u
            tps1 = ps_rt.tile([128, 128], f32, tag="vdt")
            nc.tensor.transpose(tps1[0:XCOL, :], mtok[:], IDN[:])
            mtokT = rt.tile([128, 128], f32)
            nc.vector.tensor_copy(mtokT[0:XCOL, :], tps1[0:XCOL, :])
            nc.scalar.dma_start(
                Vd1q[:].rearrange("(q t u) -> t u q", t=XCOL, u=8),
                mtokT[0:XCOL, :].rearrange("t (u q) -> t u q", q=16))

            # dummy indirect gather gated on mtokT: re-warm the dynamic
            # queue just before the real gathers.
            zoff3 = cp.tile([128, 1], i32)
            nc.vector.tensor_scalar(out=zoff3[:], in0=mtokT[:, 0:1],
                                    scalar1=0.0, scalar2=None, op0=OP.mult)
            dumg3 = cp.tile([128, 64], f16)
            nc.gpsimd.indirect_dma_start(
                out=dumg3[:], out_offset=None, in_=x16_ap[:, 0:64],
                in_offset=bass.IndirectOffsetOnAxis(ap=zoff3[:], axis=0))

            pe_warm(8)
            ps_rt_cm.__exit__(None, None, None)
            V1 = rt.tile([16, F_IN], f32)
            nc.sync.dma_start(V1[:], Vd1q[:].rearrange("(q f) -> q f", q=16))
            SG1 = rt.tile([16, F_OUT], f32)
            NF1 = rt.tile([1, 1], u32)
            nc.gpsimd.sparse_gather(SG1[:], V1[:], num_found=NF1[:])

            # The device-side slot order is a fixed permutation of the
            # reference slot order (sparse_gather's native [16,F'] order
            # read back contiguously). A, G and EOT all use the same
            # permutation, and the host aligns them by index, so this is
            # transparent -- and every hop here is contiguous.
            A_fq = dram.tile([C], f32)          # layout: q*64 + f
            nc.scalar.dma_start(A_fq[:].rearrange("(q f) -> q f", q=16), SG1[:])
            p_slot = rt.tile([128, C // 128], f32)
            nc.sync.dma_start(p_slot[:],
                              A_fq[:].rearrange("(p j) -> p j", p=128))
            p_i = rt.tile([128, C // 128], i32)
            nc.vector.tensor_copy(p_i[:], p_slot[:])
            a_i = rt.tile([128, C // 128], i32)
            nc.vector.tensor_scalar(out=a_i[:], in0=p_i[:], scalar1=11,
                                    scalar2=None, op0=OP.arith_shift_right)
            nc.scalar.dma_start(a_ap.rearrange("(s p) o -> p (s o)", p=128), a_i[:])
            gq_i = rt.tile([128, C // 128], i32)
            nc.vector.tensor_scalar(out=gq_i[:], in0=p_i[:], scalar1=2047,
                                    scalar2=None, op0=OP.bitwise_and)
            g_sb = rt.tile([128, C // 128], f32)
            nc.vector.tensor_copy(g_sb[:], gq_i[:])
            nc.vector.tensor_scalar_mul(g_sb[:], g_sb[:], 1.0 / 2047.0)
            nc.scalar.dma_start(g_ap.rearrange("(s p) o -> p (s o)", p=128), g_sb[:])
            G_f = dram.tile([C], f32)
            nc.scalar.dma_start(G_f[:].rearrange("(s p) -> p s", p=128), g_sb[:])

            # ---------------- phase 3: dispatch gather + transpose --------
            # gathers from fp16 x copy; PE transposes in fp16 (single pass)
            # g_sb-anchored warm burst: ~3.5us of PE busy right before the
            # transposes so they and GEMM1 start at full clock.
            warm_anchor(g_sb[:, 0:8], 56)
            dispTs = [bigp.tile([128, (M // 128) * CBLK], f16, name=f"dispT{c}")
                      for c in range(NCB)]  # [p, (mc, c_in_blk)]
            warm_close()
            ps_tr_cm = tc.tile_pool(name="ps_tr", bufs=2, space="PSUM")
            ps_tr = ps_tr_cm.__enter__()
            dgs = []

            def gather_sg(sg):
                dg = wk.tile([128, M], f16, tag="dg", bufs=3)
                nc.gpsimd.indirect_dma_start(
                    out=dg[:], out_offset=None, in_=x16_ap,
                    in_offset=bass.IndirectOffsetOnAxis(
                        ap=a_i[:, sg:sg + 1], axis=0))
                return dg

            def transpose_sg(sg, dg):
                cbb, sgo = divmod(sg, CBLK // 128)
                for mc in range(M // 128):
                    tp = ps_tr.tile([128, 128], f16, tag="tr")
                    nc.tensor.transpose(tp[:], dg[:, mc * 128:(mc + 1) * 128],
                                        IDN16[:])
                    nc.vector.tensor_copy(
                        dispTs[cbb][:, mc * CBLK + sgo * 128:
                                    mc * CBLK + (sgo + 1) * 128],
                        tp[:])

            # issue all gathers; transpose first c-block's 4 groups now,
            # the rest are interleaved into GEMM1 below so the tensor queue
            # stays available.
            for sg in range(C // 128):
                dgs.append(gather_sg(sg))
            for sg in range(4):
                transpose_sg(sg, dgs[sg])

            # ---------------- phases 4+5: expert FFN ----------------
            ps_ffn_cm = tc.tile_pool(name="ps_ffn", bufs=1, space="PSUM")
            ps_ffn = ps_ffn_cm.__enter__()
            hTs = [bigp.tile([128, (H // 128) * CBLK], f16, name=f"hT{c}")
                   for c in range(NCB)]
            w1v = w1_sb[:].rearrange("p (mc h) -> p mc h", h=H)
            for cb in range(NCB):
                hT = hTs[cb]
                # GEMM1: hT[h, c] = relu(w1.T @ dispT + b1)
                for htb in range(H // CBLK):              # 8 blocks of 4 ht
                    for hi in range(CBLK // 128):         # 4 ht per block
                        ht_i = htb * (CBLK // 128) + hi
                        ps1 = ps_ffn.tile([128, CBLK], f32, tag="g1", bufs=4)
                        for mc in range(M // 128):
                            nc.tensor.matmul(
                                ps1[:],
                                w1v[:, mc,
                                    htb * CBLK + hi * 128:
                                    htb * CBLK + (hi + 1) * 128],
                                dispTs[cb][:, mc * CBLK:(mc + 1) * CBLK],
                                start=(mc == 0), stop=(mc == M // 128 - 1))
                        nc.scalar.activation(
                            hT[:, ht_i * CBLK:(ht_i + 1) * CBLK], ps1[:],
                            ACTF.Relu, bias=b1c[:, ht_i:ht_i + 1], scale=1.0)
                    if cb == 0 and htb < 4:
                        # second c-block's transposes, interleaved
                        transpose_sg(4 + htb, dgs[4 + htb])

                # gate row broadcast for this c-block
                g_row = wk.tile([1, CBLK], f32, tag="grow")
                nc.sync.dma_start(g_row[0:1, :],
                                  G_f[cb * CBLK:(cb + 1) * CBLK])
                gb_ps = ps_ffn.tile([128, CBLK], f32, tag="g1", bufs=4)
                nc.tensor.matmul(gb_ps[:], ones_row[:], g_row[:],
                                 start=True, stop=True)
                g_bc = wk.tile([128, CBLK], f32, tag="gbc")
                nc.vector.tensor_copy(g_bc[:], gb_ps[:])

                # GEMM2: eoT[m, c] = w2.T @ hT ; then (+b2) * gate
                for mt in range(M // 128):
                    w2ts = wk.tile([128, H], f16, tag="w2ts", bufs=2)
                    nc.scalar.dma_start(
                        w2ts[:], w2t_ap[mt].rearrange("p (hc m) -> p hc m", m=128))
                    ps2 = ps_ffn.tile([128, CBLK], f32, tag="g2", bufs=2)
                    for hc in range(H // 128):
                        nc.tensor.matmul(
                            ps2[:], w2ts[:, hc * 128:(hc + 1) * 128],
                            hT[:, hc * CBLK:(hc + 1) * CBLK],
                            start=(hc == 0), stop=(hc == H // 128 - 1))
                    eo_sb = wk.tile([128, CBLK], f16, tag="eo")
                    nc.vector.tensor_scalar(out=eo_sb[:], in0=ps2[:],
                                            scalar1=b2c[:, mt:mt + 1],
                                            scalar2=None, op0=OP.add)
                    nc.vector.tensor_mul(eo_sb[:], eo_sb[:], g_bc[:])
                    nc.sync.dma_start(
                        eot_ap[mt * 128:(mt + 1) * 128,
                               cb * CBLK:(cb + 1) * CBLK], eo_sb[:])

            ps_ffn_cm.__exit__(None, None, None)
            ps_tr_cm.__exit__(None, None, None)
            wk_cm.__exit__(None, None, None)
            bigp_cm.__exit__(None, None, None)

    nc.compile()
    _split_multi_waits(nc)
    return nc


_NC_CACHE = None


def _get_nc():
    global _NC_CACHE
    if _NC_CACHE is None:
        _NC_CACHE = build()
    return _NC_CACHE


def _make_in_maps(x, wg, w1, b1, w2, b2):
    x2 = np.ascontiguousarray(np.asarray(x, np.float32).reshape(S, M))
    x16 = np.ascontiguousarray(x2.astype(np.float16))
    wg = np.ascontiguousarray(np.asarray(wg, np.float32))
    w1 = np.asarray(w1, np.float32)
    b1 = np.asarray(b1, np.float32)
    w2 = np.asarray(w2, np.float32)
    b2 = np.asarray(b2, np.float32)
    in_maps = []
    for k in range(N_CORES):
        shard = x2[k * (S // N_CORES):(k + 1) * (S // N_CORES)]
        xts = np.ascontiguousarray(shard.T)                    # [M, S/8]
        w1k = np.ascontiguousarray(w1[k]).astype(np.float16)   # [M, H]
        b1ck = np.ascontiguousarray(b1[k].reshape(H // 128, 128).T)
        w2k = w2[k]                                            # [H, M]
        w2t = np.ascontiguousarray(
            w2k.reshape(H // 128, 128, M // 128, 128).transpose(2, 1, 0, 3)
        ).astype(np.float16)
        b2ck = np.ascontiguousarray(b2[k].reshape(M // 128, 128).T)
        eid = np.full((128, 1), k, np.float32)
        in_maps.append({
            "x16": x16, "xts": xts, "wg": wg, "w1": w1k, "b1c": b1ck,
            "w2t": w2t, "b2c": b2ck, "eid": eid,
        })
    return in_maps


def run_cores(x, wg, w1, b1, w2, b2, trace=False, tmpdir=None):
    nc = _get_nc()
    in_maps = _make_in_maps(x, wg, w1, b1, w2, b2)
    return run_bass_kernel_spmd(nc, in_maps, list(range(N_CORES)), trace=trace,
                                tmpdir=tmpdir)


def combine(results):
    out = np.zeros((S, M), np.float32)
    for k in range(N_CORES):
        r = results[k]
        eo = np.ascontiguousarray(r["EOT"].astype(np.float32).T)  # [C, M]
        A = r["A"][:, 0].astype(np.int64)
        G = r["G"][:, 0]
        valid = G > 0
        out[A[valid]] = eo[valid]
    return out.reshape(B, SQ, M)


def kernel(x, wg, w1, b1, w2, b2):
    res = run_cores(x, wg, w1, b1, w2, b2, trace=False)
    return combine(res.results)


# revision 27
# speedup vs baseline: 1.1216x; 1.1216x over previous
"""MoE (top-1, capacity_factor=1) Trainium2 Bass kernel, expert-parallel over
8 NeuronCores. Self-contained: imports only numpy + concourse (/opt).

Per-core k (expert k resident):
  gate GEMM (fp32 exact) on its 1/8 token shard -> AllGather logits ->
  replicated routing (one-hot, global cumsum via triangular matmuls,
  capacity mask) -> slot->token table via gpsimd sparse_gather ->
  indirect row-gather of x (fp16) + PE transposes -> dispT [m, c] ->
  GEMM1 (fp16, w1 resident in SBUF) -> hT [h, c] with fused ReLU+b1 ->
  GEMM2 (fp16, w2 streamed) -> eoT [m, c], + b2 + gate scaling -> EOT (f16).
Host: scatter rows eo = EOT.T back by token id (A table), empties G==0.

v2: PE kept warm from t=0; dummy collective to absorb CC-stack warmup;
SBUF->SBUF relayouts instead of DRAM round-trips where legal; strided
loads split across sync/scalar queues; x gathered as fp16 (half bytes);
w1 fully resident; EOT in fp16.
"""
import sys

sys.path.insert(0, '/opt/trn_rl_repo')

import numpy as np
import concourse.bass as bass
import concourse.tile as tile
import concourse.mybir as mybir
from concourse import bacc
from concourse.bass_utils import run_bass_kernel_spmd
from concourse.masks import make_upper_triangular, make_identity

N_CORES = 8
B, SQ, M, E, H = 4, 2048, 1024, 8, 4096
S = B * SQ            # 8192 tokens
C = S // E            # 1024 capacity
NT = S // 128         # 64 token tiles
NTS = NT // N_CORES   # 8 token tiles per core shard
CBLK = 512            # c-block for GEMM1/GEMM2
NCB = C // CBLK       # 2 c-blocks
F_IN = (S + C) // 16  # 576  sparse_gather input free size
F_OUT = C // 16       # 64

f32 = mybir.dt.float32
f16 = mybir.dt.float16
bf16 = mybir.dt.bfloat16
i32 = mybir.dt.int32
u32 = mybir.dt.uint32
AX = mybir.AxisListType
OP = mybir.AluOpType
ACTF = mybir.ActivationFunctionType


def _split_multi_waits(nc):
    """This walrus build accepts at most ONE sync-wait per instruction.
    Split extras into same-engine NOPs inserted just before."""
    from concourse.mybir import SyncInfo
    n = 0
    for bb in list(nc.main_func.blocks):
        insts = bb.instructions  # live shared list
        for ins in list(insts):
            si = ins.sync_info
            if si is None or len(si.on_wait) <= 1:
                continue
            waits = list(si.on_wait)
            idx = insts.index(ins)
            for j, w in enumerate(waits[:-1]):
                nop = nc.engines[ins.engine].nop(nofuse=True, hint="waitsplit")
                ni = nop.ins
                cur = nc.cur_bb.bb.instructions
                if ni in cur:
                    cur.remove(ni)
                ni.sync_info = SyncInfo(on_wait=[w], on_update=[])
                insts.insert(idx + j, ni)
                n += 1
            ins.sync_info = SyncInfo(on_wait=[waits[-1]], on_update=si.on_update)
    return n


def r3(ap, e=E):
    return ap.rearrange("p (t e) -> p t e", e=e)


def build():
    nc = bacc.Bacc("TRN2", target_bir_lowering=False, debug=False,
                   num_devices=N_CORES)

    x16_ap = nc.dram_tensor("x16", [S, M], f16, kind="ExternalInput").ap()
    xts_ap = nc.dram_tensor("xts", [M, S // N_CORES], f32, kind="ExternalInput").ap()
    wg_ap = nc.dram_tensor("wg", [M, E], f32, kind="ExternalInput").ap()
    w1_ap = nc.dram_tensor("w1", [M, H], f16, kind="ExternalInput").ap()
    b1c_ap = nc.dram_tensor("b1c", [128, H // 128], f32, kind="ExternalInput").ap()
    w2t_ap = nc.dram_tensor("w2t", [M // 128, 128, H], f16, kind="ExternalInput").ap()
    b2c_ap = nc.dram_tensor("b2c", [128, M // 128], f32, kind="ExternalInput").ap()
    eid_ap = nc.dram_tensor("eid", [128, 1], f32, kind="ExternalInput").ap()

    eot_ap = nc.dram_tensor("EOT", [M, C], f16, kind="ExternalOutput").ap()
    a_ap = nc.dram_tensor("A", [C, 1], i32, kind="ExternalOutput").ap()
    g_ap = nc.dram_tensor("G", [C, 1], f32, kind="ExternalOutput").ap()

    with tile.TileContext(nc) as tc:
        with tc.tile_pool(name="consts", bufs=1) as cp, \
             tc.tile_pool(name="rt", bufs=1) as rt, \
             tc.tile_pool(name="dram", bufs=1, space="DRAM") as dram:

            # gate inputs first: these DMAs gate the whole pipeline.
            # xts strips split across both HWDGE trigger queues.
            wg_sb = cp.tile([128, (M // 128) * E], f32)   # [p, (mc, e)]
            nc.sync.dma_start(wg_sb[:].rearrange("p (t e) -> p t e", e=E),
                              wg_ap.rearrange("(mc p) e -> p mc e", p=128))
            xts_pool_cm = tc.tile_pool(name="xts", bufs=1)
            xp = xts_pool_cm.__enter__()
            xts_strips = []
            for mc in range(M // 128):
                st = xp.tile([128, S // N_CORES], f32, tag=f"xts{mc}",
                             name=f"xts{mc}")
                eng = nc.sync if mc % 2 == 0 else nc.scalar
                eng.dma_start(st[:], xts_ap[mc * 128:(mc + 1) * 128, :])
                xts_strips.append(st)

            # w1 fully resident in SBUF. The DMAs are gated behind the gate
            # GEMM output (WAW on w1_sb via the seed copy below) so the 8 MiB
            # stream does not compete with the latency-critical xts load.
            w1_sb = cp.tile([128, (M // 128) * H], f16)   # [p, (mc, h)]

            # ---------------- constants ----------------
            TRI = cp.tile([128, 128], f32)       # TRI[q,p]=1 iff q<=p
            make_upper_triangular(nc, TRI[:], val=1.0, diag=True)
            TRIS = cp.tile([64, 64], f32)        # strict upper
            make_upper_triangular(nc, TRIS[:], val=1.0, diag=False)
            IDN = cp.tile([128, 128], f32)
            make_identity(nc, IDN[:])
            IDN16 = cp.tile([128, 128], f16)
            make_identity(nc, IDN16[:])
            ones_col = cp.tile([128, 1], f32)
            nc.vector.memset(ones_col[:], 1.0)
            ones_row = cp.tile([1, 128], f32)
            nc.vector.memset(ones_row[:], 1.0)
            iota_e = cp.tile([128, NT * E], f32)
            nc.gpsimd.iota(r3(iota_e[:]), pattern=[[0, NT], [1, E]], base=0,
                           channel_multiplier=0,
                           allow_small_or_imprecise_dtypes=True)
            tokf = cp.tile([128, NT], f32)
            nc.gpsimd.iota(tokf[:], pattern=[[128, NT]], base=0,
                           channel_multiplier=1,
                           allow_small_or_imprecise_dtypes=True)
            eid_sb = cp.tile([128, 1], f32)
            nc.scalar.dma_start(eid_sb[:], eid_ap[:])
            b1c = cp.tile([128, H // 128], f32)
            nc.scalar.dma_start(b1c[:], b1c_ap[:])
            b2c = cp.tile([128, M // 128], f32)
            nc.scalar.dma_start(b2c[:], b2c_ap[:])

            # preload the sparse_gather ucode library early
            dumv = cp.tile([16, 8], f32)
            nc.vector.memset(dumv[:], 1.0)
            dumo = cp.tile([16, 8], f32)
            dumn = cp.tile([1, 1], u32)
            nc.gpsimd.sparse_gather(dumo[:], dumv[:], num_found=dumn[:])
            zoff = cp.tile([128, 1], i32)
            nc.vector.memset(zoff[:], 0)
            dumg = cp.tile([128, 64], f16)
            nc.gpsimd.indirect_dma_start(
                out=dumg[:], out_offset=None, in_=x16_ap[:, 0:64],
                in_offset=bass.IndirectOffsetOnAxis(ap=zoff[:], axis=0))

            # PE warm machinery: keep the HAM activity monitor fed so real
            # matmuls run at full clock. PSUM banks are pool-granular, so the
            # warm pool opens/closes around phases that need all 8 banks.
            warm_in = cp.tile([128, 64], bf16)
            nc.vector.memset(warm_in[:], 1.0)
            warm_w = cp.tile([128, 64], bf16)
            nc.vector.memset(warm_w[:], 1.0)
            warm_state = {}

            def warm_open():
                cm = tc.tile_pool(name="warm_ps", bufs=1, space="PSUM")
                pool = cm.__enter__()
                warm_state['cm'] = cm
                # 4 round-robin output tiles: WAW deps on a single tile
                # would serialize the warm matmuls into isolated (drain-
                # latency) issues and clog the tensor queue.
                warm_state['outs'] = [
                    pool.tile([64, 64], f32, tag=f"warm{j}", name=f"warm{j}")
                    for j in range(4)]
                warm_state['rr'] = 0

            def warm_close():
                warm_state.pop('cm').__exit__(None, None, None)

            def pe_warm(n):
                for _ in range(n):
                    j = warm_state['rr'] = (warm_state['rr'] + 1) % 4
                    nc.tensor.matmul(warm_state['outs'][j][:], warm_w[:],
                                     warm_in[:], start=True, stop=True,
                                     skip_group_check=True)

            warm_in32 = cp.tile([128, 64], f32)
            nc.vector.memset(warm_in32[:], 1.0)

            def warm_anchor(src_ap, np_):
                # a matmul that READS src_ap chains the warm burst behind
                # that tile's producer (the scheduler reorders freely, so
                # data deps are the only reliable sequencing tool).
                nc.tensor.matmul(warm_state['outs'][0][0:src_ap.shape[1], 0:64],
                                 src_ap, warm_in32[:, 0:64],
                                 start=True, stop=True, skip_group_check=True)
                pe_warm(np_)

            # ~4us of initial PE busy to trip the HAM SHORT window before
            # the gate GEMM begins.
            warm_open()
            pe_warm(64)
            warm_close()

            # ---------------- phase 1: gate GEMM (fp32) ----------------
            Lg_sb = rt.tile([128, NTS * E], f32)          # own shard logits
            Lg_in = dram.tile([S // N_CORES], f32)        # packed eidx+gv/2
            Lg_all = dram.tile([S], f32, addr_space="Shared")

            with tc.tile_pool(name="psg_p", bufs=1, space="PSUM") as psg_p:
                psgs = [psg_p.tile([128, E], f32, tag=f"psg{t}", name=f"psg{t}")
                        for t in range(NTS)]
                for mc in range(M // 128):
                    st = xts_strips[mc]
                    for t in range(NTS):
                        nc.tensor.matmul(
                            psgs[t][:], st[:, t * 128:(t + 1) * 128],
                            wg_sb[:, mc * E:(mc + 1) * E],
                            start=(mc == 0), stop=(mc == M // 128 - 1))
                for t in range(NTS):
                    nc.vector.tensor_copy(Lg_sb[:, t * E:(t + 1) * E], psgs[t][:])
            xts_pool_cm.__exit__(None, None, None)

            # -------- local shard routing (softmax/argmax) pre-AllGather,
            # AllGather ships one packed fp32 per token: v = eidx + gv/2.
            # eidx in [0,8), gv in (1/8, 1): frac(v) = gv/2 in (1/16, 1/2),
            # so int-copy recovers eidx under truncation AND round-nearest.
            Lg3 = Lg_sb[:].rearrange("p (t e) -> p t e", e=E)
            lmax_l = rt.tile([128, NTS], f32)
            nc.vector.tensor_reduce(lmax_l[:], Lg3, axis=AX.X, op=OP.max)
            lmax_lb = lmax_l[:].rearrange("p (t o) -> p t o", o=1) \
                               .to_broadcast([128, NTS, E])
            dd_l = rt.tile([128, NTS * E], f32)
            nc.vector.tensor_tensor(out=r3(dd_l[:]), in0=Lg3, in1=lmax_lb,
                                    op=OP.subtract)
            expd_l = rt.tile([128, NTS * E], f32)
            nc.scalar.activation(expd_l[:], dd_l[:], ACTF.Exp)
            ssum_l = rt.tile([128, NTS], f32)
            nc.vector.tensor_reduce(ssum_l[:], r3(expd_l[:]), axis=AX.X,
                                    op=OP.add)
            oh_l = rt.tile([128, NTS * E], f32)
            nc.vector.tensor_tensor(out=r3(oh_l[:]), in0=Lg3, in1=lmax_lb,
                                    op=OP.is_equal)
            tmp_l = rt.tile([128, NTS * E], f32)
            nc.vector.tensor_mul(tmp_l[:], oh_l[:], iota_e[:, :NTS * E])
            v_sb = rt.tile([128, NTS], f32)
            nc.vector.tensor_reduce(v_sb[:], r3(tmp_l[:]), axis=AX.X, op=OP.add)
            gv_l = rt.tile([128, NTS], f32)
            nc.vector.reciprocal(gv_l[:], ssum_l[:])
            nc.vector.tensor_scalar(out=gv_l[:], in0=gv_l[:], scalar1=0.5,
                                    scalar2=None, op0=OP.mult)
            nc.vector.tensor_add(v_sb[:], v_sb[:], gv_l[:])

            # partition-major payload so the post-AG load moves 32B runs
            nc.sync.dma_start(Lg_in[:].rearrange("(p t) -> p t", p=128),
                              v_sb[:])
            nc.gpsimd.collective_compute(
                "AllGather", OP.bypass,
                replica_groups=[list(range(N_CORES))],
                ins=[Lg_in[:]], outs=[Lg_all[:]])
            warm_open()

            # release the w1 stream now that the gate GEMM's inputs are done:
            # seed WAW dep on w1_sb from Lg_sb, then the 4 big loads.
            nc.vector.tensor_copy(w1_sb[0:1, 0:1], Lg_sb[0:1, 0:1])
            for qq in range(4):
                nc.scalar.dma_start(
                    w1_sb[:].rearrange("p (mc h) -> p mc h", h=H)
                    [:, qq * 2:(qq + 1) * 2, :],
                    w1_ap.rearrange("(mc p) h -> p mc h", p=128)
                    [:, qq * 2:(qq + 1) * 2, :])

            # dummy indirect gather gated on Lg_sb: keeps the dynamic DMA
            # queue spun up so the real gathers don't pay cold-start.
            zoff2 = cp.tile([128, 1], i32)
            nc.vector.tensor_scalar(out=zoff2[:], in0=Lg_sb[:, 0:1],
                                    scalar1=0.0, scalar2=None, op0=OP.mult)
            dumg2 = cp.tile([128, 64], f16)
            nc.gpsimd.indirect_dma_start(
                out=dumg2[:], out_offset=None, in_=x16_ap[:, 0:64],
                in_offset=bass.IndirectOffsetOnAxis(ap=zoff2[:], axis=0))

            bigp_cm = tc.tile_pool(name="big", bufs=1)
            bigp = bigp_cm.__enter__()
            wk_cm = tc.tile_pool(name="wk", bufs=2)
            wk = wk_cm.__enter__()

            # ---------------- phase 2: routing (replicated) ----------------
            # load the packed payload and decode eidx / gv
            L4 = rt.tile([128, NT], f32)
            nc.sync.dma_start(
                L4[:].rearrange("p (c t) -> p c t", t=NTS),
                Lg_all[:].rearrange("(c p t) -> p c t", p=128, t=NTS))
            warm_anchor(L4[:, 0:8], 44)

            ei32 = rt.tile([128, NT], i32)
            nc.vector.tensor_copy(ei32[:], L4[:])
            ef = rt.tile([128, NT], f32)
            nc.vector.tensor_copy(ef[:], ei32[:])
            gv = rt.tile([128, NT], f32)
            nc.vector.tensor_tensor(out=gv[:], in0=L4[:], in1=ef[:],
                                    op=OP.subtract)
            nc.vector.tensor_scalar(out=gv[:], in0=gv[:], scalar1=2.0,
                                    scalar2=None, op0=OP.mult)
            ef_b = ef[:].rearrange("p (t o) -> p t o", o=1) \
                        .to_broadcast([128, NT, E])
            oh = rt.tile([128, NT * E], f32)
            nc.vector.tensor_tensor(out=r3(oh[:]), in0=r3(iota_e[:]), in1=ef_b,
                                    op=OP.is_equal)

            ps_rt_cm = tc.tile_pool(name="ps_rt", bufs=1, space="PSUM")
            ps_rt = ps_rt_cm.__enter__()
            pos_ps = ps_rt.tile([128, NT * E], f32, tag="pos")
            nc.tensor.matmul(pos_ps[:], TRI[:], oh[:], start=True, stop=False)
            # per-(tile,expert) counts in one matmul: ones_col.T @ oh
            cnt1_ps = ps_rt.tile([1, NT * E], f32, tag="misc", name="cnt1")
            nc.tensor.matmul(cnt1_ps[:], ones_col[:], oh[:],
                             start=True, stop=True)
            cnt_sb = rt.tile([1, NT * E], f32)
            nc.vector.tensor_copy(cnt_sb[:], cnt1_ps[:])
            cnt64 = rt.tile([64, 8], f32)
            nc.scalar.dma_start(cnt64[:], cnt_sb[0:1, :])
            offs_ps = ps_rt.tile([64, 8], f32, tag="misc", name="offsps")
            nc.tensor.matmul(offs_ps[:], TRIS[:], cnt64[:], start=True, stop=True)
            offs_sb = rt.tile([64, 8], f32)
            nc.vector.tensor_copy(offs_sb[:], offs_ps[:])
            # direct SBUF->SBUF relayout [64p,8f] -> [1p,512f] (row-major both)
            offs_flat = rt.tile([1, NT * E], f32)
            nc.scalar.dma_start(offs_flat[0:1, :], offs_sb[:])
            nc.tensor.matmul(pos_ps[:], ones_row[:], offs_flat[:],
                             start=False, stop=True)

            pm = rt.tile([128, NT * E], f32)
            nc.vector.tensor_mul(pm[:], pos_ps[:], oh[:])
            pos_tok = rt.tile([128, NT], f32)
            nc.vector.tensor_reduce(pos_tok[:], r3(pm[:]), axis=AX.X, op=OP.add)
            nc.vector.tensor_scalar_add(pos_tok[:], pos_tok[:], -1.0)

            keep = rt.tile([128, NT], f32)
            nc.vector.tensor_scalar(out=keep[:], in0=pos_tok[:],
                                    scalar1=float(C), scalar2=None, op0=OP.is_lt)
            mine = rt.tile([128, NT], f32)
            nc.vector.tensor_scalar(out=mine[:], in0=ef[:],
                                    scalar1=eid_sb[:, 0:1], scalar2=None,
                                    op0=OP.is_equal)
            sel = rt.tile([128, NT], f32)
            nc.vector.tensor_mul(sel[:], mine[:], keep[:])
            gvk = rt.tile([128, NT], f32)
            nc.vector.tensor_mul(gvk[:], gv[:], keep[:])

            # packed payload: tokid*2048 + round(gv*2047) if sel else -1;
            # cols NT..NT+7 stay 0 (always-kept sentinels -> A=0, G=0)
            XCOL = NT + C // 128
            mtok = rt.tile([128, XCOL], f32)
            nc.vector.memset(mtok[:, NT:], 0.0)
            nc.vector.tensor_scalar(out=mtok[:, :NT], in0=tokf[:], scalar1=2048.0,
                                    scalar2=1.0, op0=OP.mult, op1=OP.add)
            gq = rt.tile([128, NT], f32)
            nc.vector.tensor_scalar_mul(gq[:], gvk[:], 2047.0)
            nc.vector.tensor_add(mtok[:, :NT], mtok[:, :NT], gq[:])
            nc.vector.tensor_mul(mtok[:, :NT], mtok[:, :NT], sel[:])
            nc.vector.tensor_scalar_add(mtok[:, :NT], mtok[:, :NT], -1.0)

            # ------ sparse_gather compaction: slot -> (tokid, gate) ------
            # V1 [16, 576] is built entirely in SBUF with two rounds of PE
            # transposes (a DRAM staging hop here measured 4B-packet-bound).
            # V1[q, t*8+u] = mtok[u*16+q, t].
            tps1 = ps_rt.tile([128, 128], f32, tag="vdt")
            nc.tensor.transpose(tps1[0:XCOL, :], mtok[:], IDN[:])
            mtokT = rt.tile([128, 128], f32)
            nc.vector.tensor_copy(mtokT[0:XCOL, :], tps1[0:XCOL, :])
            V1 = rt.tile([16, F_IN], f32)
            V1v = V1[:].rearrange("q (t u) -> q t u", u=8)
            for u in range(8):
                tpsV = ps_rt.tile([16, XCOL], f32, tag="tpsV", name="tpsV")
                nc.tensor.transpose(tpsV[:], mtokT[0:XCOL, u * 16:(u + 1) * 16],
                                    IDN[0:XCOL, 0:XCOL])
                nc.vector.tensor_copy(V1v[:, :, u], tpsV[:])

            # dummy indirect gather gated on mtokT: re-warm the dynamic
            # queue just before the real gathers.
            zoff3 = cp.tile([128, 1], i32)
            nc.vector.tensor_scalar(out=zoff3[:], in0=mtokT[:, 0:1],
                                    scalar1=0.0, scalar2=None, op0=OP.mult)
            dumg3 = cp.tile([128, 64], f16)
            nc.gpsimd.indirect_dma_start(
                out=dumg3[:], out_offset=None, in_=x16_ap[:, 0:64],
                in_offset=bass.IndirectOffsetOnAxis(ap=zoff3[:], axis=0))

            ps_rt_cm.__exit__(None, None, None)
            SG1 = rt.tile([16, F_OUT], f32)
            NF1 = rt.tile([1, 1], u32)
            nc.gpsimd.sparse_gather(SG1[:], V1[:], num_found=NF1[:])

            # The device-side slot order is a fixed permutation of the
            # reference slot order (sparse_gather's native [16,F'] order
            # read back contiguously). A, G and EOT all use the same
            # permutation, and the host aligns them by index, so this is
            # transparent -- and every hop here is contiguous.
            A_fq = dram.tile([C], f32)          # layout: q*64 + f
            nc.scalar.dma_start(A_fq[:].rearrange("(q f) -> q f", q=16), SG1[:])
            p_slot = rt.tile([128, C // 128], f32)
            nc.sync.dma_start(p_slot[:],
                              A_fq[:].rearrange("(p j) -> p j", p=128))
            p_i = rt.tile([128, C // 128], i32)
            nc.vector.tensor_copy(p_i[:], p_slot[:])
            a_i = rt.tile([128, C // 128], i32)
            nc.vector.tensor_scalar(out=a_i[:], in0=p_i[:], scalar1=11,
                                    scalar2=None, op0=OP.arith_shift_right)
            nc.scalar.dma_start(a_ap.rearrange("(s p) o -> p (s o)", p=128), a_i[:])
            gq_i = rt.tile([128, C // 128], i32)
            nc.vector.tensor_scalar(out=gq_i[:], in0=p_i[:], scalar1=2047,
                                    scalar2=None, op0=OP.bitwise_and)
            g_sb = rt.tile([128, C // 128], f32)
            nc.vector.tensor_copy(g_sb[:], gq_i[:])
            nc.vector.tensor_scalar_mul(g_sb[:], g_sb[:], 1.0 / 2047.0)
            nc.scalar.dma_start(g_ap.rearrange("(s p) o -> p (s o)", p=128), g_sb[:])
            G_f = dram.tile([C], f32)
            nc.scalar.dma_start(G_f[:].rearrange("(s p) -> p s", p=128), g_sb[:])

            # ---------------- phase 3: dispatch gather + transpose --------
            # gathers from fp16 x copy; PE transposes in fp16 (single pass)
            # g_sb-anchored warm burst: ~3.5us of PE busy right before the
            # transposes so they and GEMM1 start at full clock.
            warm_anchor(g_sb[:, 0:8], 56)
            dispTs = [bigp.tile([128, (M // 128) * CBLK], f16, name=f"dispT{c}")
                      for c in range(NCB)]  # [p, (mc, c_in_blk)]
            warm_close()
            ps_tr_cm = tc.tile_pool(name="ps_tr", bufs=2, space="PSUM")
            ps_tr = ps_tr_cm.__enter__()
            dgs = []

            def gather_sg(sg):
                dg = wk.tile([128, M], f16, tag="dg", bufs=3)
                nc.gpsimd.indirect_dma_start(
                    out=dg[:], out_offset=None, in_=x16_ap,
                    in_offset=bass.IndirectOffsetOnAxis(
                        ap=a_i[:, sg:sg + 1], axis=0))
                return dg

            def transpose_sg(sg, dg):
                cbb, sgo = divmod(sg, CBLK // 128)
                for mc in range(M // 128):
                    tp = ps_tr.tile([128, 128], f16, tag="tr")
                    nc.tensor.transpose(tp[:], dg[:, mc * 128:(mc + 1) * 128],
                                        IDN16[:])
                    nc.vector.tensor_copy(
                        dispTs[cbb][:, mc * CBLK + sgo * 128:
                                    mc * CBLK + (sgo + 1) * 128],
                        tp[:])

            # issue all gathers; transpose first c-block's 4 groups now,
            # the rest are interleaved into GEMM1 below so the tensor queue
            # stays available.
            for sg in range(C // 128):
                dgs.append(gather_sg(sg))
            for sg in range(4):
                transpose_sg(sg, dgs[sg])

            # ---------------- phases 4+5: expert FFN ----------------
            ps_ffn_cm = tc.tile_pool(name="ps_ffn", bufs=1, space="PSUM")
            ps_ffn = ps_ffn_cm.__enter__()
            hTs = [bigp.tile([128, (H // 128) * CBLK], f16, name=f"hT{c}")
                   for c in range(NCB)]
            w1v = w1_sb[:].rearrange("p (mc h) -> p mc h", h=H)
            for cb in range(NCB):
                hT = hTs[cb]
                # GEMM1: hT[h, c] = relu(w1.T @ dispT + b1)
                for htb in range(H // CBLK):              # 8 blocks of 4 ht
                    for hi in range(CBLK // 128):         # 4 ht per block
                        ht_i = htb * (CBLK // 128) + hi
                        ps1 = ps_ffn.tile([128, CBLK], f32, tag="g1", bufs=4)
                        for mc in range(M // 128):
                            nc.tensor.matmul(
                                ps1[:],
                                w1v[:, mc,
                                    htb * CBLK + hi * 128:
                                    htb * CBLK + (hi + 1) * 128],
                                dispTs[cb][:, mc * CBLK:(mc + 1) * CBLK],
                                start=(mc == 0), stop=(mc == M // 128 - 1))
                        nc.scalar.activation(
                            hT[:, ht_i * CBLK:(ht_i + 1) * CBLK], ps1[:],
                            ACTF.Relu, bias=b1c[:, ht_i:ht_i + 1], scale=1.0)
                    if cb == 0 and htb < 4:
                        # second c-block's transposes, interleaved
                        transpose_sg(4 + htb, dgs[4 + htb])

                # gate row broadcast for this c-block
                g_row = wk.tile([1, CBLK], f32, tag="grow")
                nc.sync.dma_start(g_row[0:1, :],
                                  G_f[cb * CBLK:(cb + 1) * CBLK])
                gb_ps = ps_ffn.tile([128, CBLK], f32, tag="g1", bufs=4)
                nc.tensor.matmul(gb_ps[:], ones_row[:], g_row[:],
                                 start=True, stop=True)
                g_bc = wk.tile([128, CBLK], f32, tag="gbc")
                nc.vector.tensor_copy(g_bc[:], gb_ps[:])

                # GEMM2: eoT[m, c] = w2.T @ hT ; then (+b2) * gate
                for mt in range(M // 128):
                    w2ts = wk.tile([128, H], f16, tag="w2ts", bufs=2)
                    nc.scalar.dma_start(
                        w2ts[:], w2t_ap[mt].rearrange("p (hc m) -> p hc m", m=128))
                    ps2 = ps_ffn.tile([128, CBLK], f32, tag="g2", bufs=2)
                    for hc in range(H // 128):
                        nc.tensor.matmul(
                            ps2[:], w2ts[:, hc * 128:(hc + 1) * 128],
                            hT[:, hc * CBLK:(hc + 1) * CBLK],
                            start=(hc == 0), stop=(hc == H // 128 - 1))
                    eo_sb = wk.tile([128, CBLK], f16, tag="eo")
                    nc.vector.tensor_scalar(out=eo_sb[:], in0=ps2[:],
                                            scalar1=b2c[:, mt:mt + 1],
                                            scalar2=None, op0=OP.add)
                    nc.vector.tensor_mul(eo_sb[:], eo_sb[:], g_bc[:])
                    nc.sync.dma_start(
                        eot_ap[mt * 128:(mt + 1) * 128,
                               cb * CBLK:(cb + 1) * CBLK], eo_sb[:])

            ps_ffn_cm.__exit__(None, None, None)
            ps_tr_cm.__exit__(None, None, None)
            wk_cm.__exit__(None, None, None)
            bigp_cm.__exit__(None, None, None)

    nc.compile()
    _split_multi_waits(nc)
    return nc


_NC_CACHE = None


def _get_nc():
    global _NC_CACHE
    if _NC_CACHE is None:
        _NC_CACHE = build()
    return _NC_CACHE


def _make_in_maps(x, wg, w1, b1, w2, b2):
    x2 = np.ascontiguousarray(np.asarray(x, np.float32).reshape(S, M))
    x16 = np.ascontiguousarray(x2.astype(np.float16))
    wg = np.ascontiguousarray(np.asarray(wg, np.float32))
    w1 = np.asarray(w1, np.float32)
    b1 = np.asarray(b1, np.float32)
    w2 = np.asarray(w2, np.float32)
    b2 = np.asarray(b2, np.float32)
    in_maps = []
    for k in range(N_CORES):
        shard = x2[k * (S // N_CORES):(k + 1) * (S // N_CORES)]
        xts = np.ascontiguousarray(shard.T)                    # [M, S/8]
        w1k = np.ascontiguousarray(w1[k]).astype(np.float16)   # [M, H]
        b1ck = np.ascontiguousarray(b1[k].reshape(H // 128, 128).T)
        w2k = w2[k]                                            # [H, M]
        w2t = np.ascontiguousarray(
            w2k.reshape(H // 128, 128, M // 128, 128).transpose(2, 1, 0, 3)
        ).astype(np.float16)
        b2ck = np.ascontiguousarray(b2[k].reshape(M // 128, 128).T)
        eid = np.full((128, 1), k, np.float32)
        in_maps.append({
            "x16": x16, "xts": xts, "wg": wg, "w1": w1k, "b1c": b1ck,
            "w2t": w2t, "b2c": b2ck, "eid": eid,
        })
    return in_maps


def run_cores(x, wg, w1, b1, w2, b2, trace=False, tmpdir=None):
    nc = _get_nc()
    in_maps = _make_in_maps(x, wg, w1, b1, w2, b2)
    return run_bass_kernel_spmd(nc, in_maps, list(range(N_CORES)), trace=trace,
                                tmpdir=tmpdir)


def combine(results):
    out = np.zeros((S, M), np.float32)
    for k in range(N_CORES):
        r = results[k]
        eo = np.ascontiguousarray(r["EOT"].astype(np.float32).T)  # [C, M]
        A = r["A"][:, 0].astype(np.int64)
        G = r["G"][:, 0]
        valid = G > 0
        out[A[valid]] = eo[valid]
    return out.reshape(B, SQ, M)


def kernel(x, wg, w1, b1, w2, b2):
    res = run_cores(x, wg, w1, b1, w2, b2, trace=False)
    return combine(res.results)


# revision 29
# speedup vs baseline: 1.1364x; 1.0132x over previous
"""MoE (top-1, capacity_factor=1) Trainium2 Bass kernel, expert-parallel over
8 NeuronCores. Self-contained: imports only numpy + concourse (/opt).

Per-core k (expert k resident):
  gate GEMM (fp32 exact) on its 1/8 token shard -> AllGather logits ->
  replicated routing (one-hot, global cumsum via triangular matmuls,
  capacity mask) -> slot->token table via gpsimd sparse_gather ->
  indirect row-gather of x (fp16) + PE transposes -> dispT [m, c] ->
  GEMM1 (fp16, w1 resident in SBUF) -> hT [h, c] with fused ReLU+b1 ->
  GEMM2 (fp16, w2 streamed) -> eoT [m, c], + b2 + gate scaling -> EOT (f16).
Host: scatter rows eo = EOT.T back by token id (A table), empties G==0.

v2: PE kept warm from t=0; dummy collective to absorb CC-stack warmup;
SBUF->SBUF relayouts instead of DRAM round-trips where legal; strided
loads split across sync/scalar queues; x gathered as fp16 (half bytes);
w1 fully resident; EOT in fp16.
"""
import sys

sys.path.insert(0, '/opt/trn_rl_repo')

import numpy as np
import concourse.bass as bass
import concourse.tile as tile
import concourse.mybir as mybir
from concourse import bacc
from concourse.bass_utils import run_bass_kernel_spmd
from concourse.masks import make_upper_triangular, make_identity

N_CORES = 8
B, SQ, M, E, H = 4, 2048, 1024, 8, 4096
S = B * SQ            # 8192 tokens
C = S // E            # 1024 capacity
NT = S // 128         # 64 token tiles
NTS = NT // N_CORES   # 8 token tiles per core shard
CBLK = 512            # c-block for GEMM1/GEMM2
NCB = C // CBLK       # 2 c-blocks
F_IN = (S + C) // 16  # 576  sparse_gather input free size
F_OUT = C // 16       # 64

f32 = mybir.dt.float32
f16 = mybir.dt.float16
bf16 = mybir.dt.bfloat16
i32 = mybir.dt.int32
u32 = mybir.dt.uint32
AX = mybir.AxisListType
OP = mybir.AluOpType
ACTF = mybir.ActivationFunctionType


def _split_multi_waits(nc):
    """This walrus build accepts at most ONE sync-wait per instruction.
    Split extras into same-engine NOPs inserted just before."""
    from concourse.mybir import SyncInfo
    n = 0
    for bb in list(nc.main_func.blocks):
        insts = bb.instructions  # live shared list
        for ins in list(insts):
            si = ins.sync_info
            if si is None or len(si.on_wait) <= 1:
                continue
            waits = list(si.on_wait)
            idx = insts.index(ins)
            for j, w in enumerate(waits[:-1]):
                nop = nc.engines[ins.engine].nop(nofuse=True, hint="waitsplit")
                ni = nop.ins
                cur = nc.cur_bb.bb.instructions
                if ni in cur:
                    cur.remove(ni)
                ni.sync_info = SyncInfo(on_wait=[w], on_update=[])
                insts.insert(idx + j, ni)
                n += 1
            ins.sync_info = SyncInfo(on_wait=[waits[-1]], on_update=si.on_update)
    return n


def r3(ap, e=E):
    return ap.rearrange("p (t e) -> p t e", e=e)


def build():
    nc = bacc.Bacc("TRN2", target_bir_lowering=False, debug=False,
                   num_devices=N_CORES)

    x16_ap = nc.dram_tensor("x16", [S, M], f16, kind="ExternalInput").ap()
    xts_ap = nc.dram_tensor("xts", [M, S // N_CORES], f32, kind="ExternalInput").ap()
    wg_ap = nc.dram_tensor("wg", [M, E], f32, kind="ExternalInput").ap()
    w1_ap = nc.dram_tensor("w1", [M, H], f16, kind="ExternalInput").ap()
    b1c_ap = nc.dram_tensor("b1c", [128, H // 128], f32, kind="ExternalInput").ap()
    w2t_ap = nc.dram_tensor("w2t", [M // 128, 128, H], f16, kind="ExternalInput").ap()
    b2c_ap = nc.dram_tensor("b2c", [128, M // 128], f32, kind="ExternalInput").ap()
    eid_ap = nc.dram_tensor("eid", [128, 1], f32, kind="ExternalInput").ap()

    eot_ap = nc.dram_tensor("EOT", [M, C], f16, kind="ExternalOutput").ap()
    a_ap = nc.dram_tensor("A", [C, 1], i32, kind="ExternalOutput").ap()
    g_ap = nc.dram_tensor("G", [C, 1], f32, kind="ExternalOutput").ap()

    with tile.TileContext(nc) as tc:
        with tc.tile_pool(name="consts", bufs=1) as cp, \
             tc.tile_pool(name="rt", bufs=1) as rt, \
             tc.tile_pool(name="dram", bufs=1, space="DRAM") as dram:

            # gate inputs first: these DMAs gate the whole pipeline.
            # xts strips split across both HWDGE trigger queues.
            wg_sb = cp.tile([128, (M // 128) * E], f32)   # [p, (mc, e)]
            nc.sync.dma_start(wg_sb[:].rearrange("p (t e) -> p t e", e=E),
                              wg_ap.rearrange("(mc p) e -> p mc e", p=128))
            xts_pool_cm = tc.tile_pool(name="xts", bufs=1)
            xp = xts_pool_cm.__enter__()
            xts_strips = []
            for mc in range(M // 128):
                st = xp.tile([128, S // N_CORES], f32, tag=f"xts{mc}",
                             name=f"xts{mc}")
                eng = nc.sync if mc % 2 == 0 else nc.scalar
                eng.dma_start(st[:], xts_ap[mc * 128:(mc + 1) * 128, :])
                xts_strips.append(st)

            # w1 fully resident in SBUF. The DMAs are gated behind the gate
            # GEMM output (WAW on w1_sb via the seed copy below) so the 8 MiB
            # stream does not compete with the latency-critical xts load.
            w1_sb = cp.tile([128, (M // 128) * H], f16)   # [p, (mc, h)]

            # ---------------- constants ----------------
            TRI = cp.tile([128, 128], f32)       # TRI[q,p]=1 iff q<=p
            make_upper_triangular(nc, TRI[:], val=1.0, diag=True)
            TRIS = cp.tile([64, 64], f32)        # strict upper
            make_upper_triangular(nc, TRIS[:], val=1.0, diag=False)
            IDN = cp.tile([128, 128], f32)
            make_identity(nc, IDN[:])
            IDN16 = cp.tile([128, 128], f16)
            make_identity(nc, IDN16[:])
            ones_col = cp.tile([128, 1], f32)
            nc.vector.memset(ones_col[:], 1.0)
            ones_row = cp.tile([1, 128], f32)
            nc.vector.memset(ones_row[:], 1.0)
            iota_e = cp.tile([128, NT * E], f32)
            nc.gpsimd.iota(r3(iota_e[:]), pattern=[[0, NT], [1, E]], base=0,
                           channel_multiplier=0,
                           allow_small_or_imprecise_dtypes=True)
            tokf = cp.tile([128, NT], f32)
            nc.gpsimd.iota(tokf[:], pattern=[[128, NT]], base=0,
                           channel_multiplier=1,
                           allow_small_or_imprecise_dtypes=True)
            eid_sb = cp.tile([128, 1], f32)
            nc.scalar.dma_start(eid_sb[:], eid_ap[:])
            b1c = cp.tile([128, H // 128], f32)
            nc.scalar.dma_start(b1c[:], b1c_ap[:])
            b2c = cp.tile([128, M // 128], f32)
            nc.scalar.dma_start(b2c[:], b2c_ap[:])

            # preload the sparse_gather ucode library early
            dumv = cp.tile([16, 8], f32)
            nc.vector.memset(dumv[:], 1.0)
            dumo = cp.tile([16, 8], f32)
            dumn = cp.tile([1, 1], u32)
            nc.gpsimd.sparse_gather(dumo[:], dumv[:], num_found=dumn[:])
            zoff = cp.tile([128, 1], i32)
            nc.vector.memset(zoff[:], 0)
            dumg = cp.tile([128, 64], f16)
            nc.gpsimd.indirect_dma_start(
                out=dumg[:], out_offset=None, in_=x16_ap[:, 0:64],
                in_offset=bass.IndirectOffsetOnAxis(ap=zoff[:], axis=0))

            # PE warm machinery: keep the HAM activity monitor fed so real
            # matmuls run at full clock. PSUM banks are pool-granular, so the
            # warm pool opens/closes around phases that need all 8 banks.
            warm_in = cp.tile([128, 64], bf16)
            nc.vector.memset(warm_in[:], 1.0)
            warm_w = cp.tile([128, 64], bf16)
            nc.vector.memset(warm_w[:], 1.0)
            warm_state = {}

            def warm_open():
                cm = tc.tile_pool(name="warm_ps", bufs=1, space="PSUM")
                pool = cm.__enter__()
                warm_state['cm'] = cm
                # 4 round-robin output tiles: WAW deps on a single tile
                # would serialize the warm matmuls into isolated (drain-
                # latency) issues and clog the tensor queue.
                warm_state['outs'] = [
                    pool.tile([64, 64], f32, tag=f"warm{j}", name=f"warm{j}")
                    for j in range(4)]
                warm_state['rr'] = 0

            def warm_close():
                warm_state.pop('cm').__exit__(None, None, None)

            def pe_warm(n):
                for _ in range(n):
                    j = warm_state['rr'] = (warm_state['rr'] + 1) % 4
                    nc.tensor.matmul(warm_state['outs'][j][:], warm_w[:],
                                     warm_in[:], start=True, stop=True,
                                     skip_group_check=True)

            warm_in32 = cp.tile([128, 64], f32)
            nc.vector.memset(warm_in32[:], 1.0)

            def warm_anchor(src_ap, np_):
                # a matmul that READS src_ap chains the warm burst behind
                # that tile's producer (the scheduler reorders freely, so
                # data deps are the only reliable sequencing tool).
                nc.tensor.matmul(warm_state['outs'][0][0:src_ap.shape[1], 0:64],
                                 src_ap, warm_in32[:, 0:64],
                                 start=True, stop=True, skip_group_check=True)
                pe_warm(np_)

            # ~4us of initial PE busy to trip the HAM SHORT window before
            # the gate GEMM begins.
            warm_open()
            pe_warm(64)
            warm_close()

            # ---------------- phase 1: gate GEMM (fp32) ----------------
            Lg_sb = rt.tile([128, NTS * E], f32)          # own shard logits
            Lg_in = dram.tile([S // N_CORES], f32)        # packed eidx+gv/2
            Lg_all = dram.tile([S], f32, addr_space="Shared")

            # wg stationary (8-col LDWEIGHTS), tokens streaming N=512: the
            # PE runs at full duty so HAM stays warm; the original
            # token-partition orientation reloaded a 128-col stationary per
            # N=8 matmul and was LDWEIGHTS-latency-bound (~23us cold).
            with tc.tile_pool(name="psg_p", bufs=1, space="PSUM") as psg_p:
                pgs = [psg_p.tile([8, 512], f32, tag=f"pg{h}", name=f"pg{h}")
                       for h in range(2)]
                for mc in range(M // 128):
                    st = xts_strips[mc]
                    for h in range(2):
                        nc.tensor.matmul(
                            pgs[h][:], wg_sb[:, mc * E:(mc + 1) * E],
                            st[:, h * 512:(h + 1) * 512],
                            start=(mc == 0), stop=(mc == M // 128 - 1))
                Lg8 = rt.tile([8, S // N_CORES], f32)
                for h in range(2):
                    nc.vector.tensor_copy(Lg8[:, h * 512:(h + 1) * 512],
                                          pgs[h][:])
                # transpose back to [token-partition, (tile, e)]
                for t in range(NTS):
                    tg = psg_p.tile([128, E], f32, tag="tg", bufs=2, name="tg")
                    nc.tensor.transpose(tg[:], Lg8[:, t * 128:(t + 1) * 128],
                                        IDN[0:8, 0:8])
                    nc.vector.tensor_copy(Lg_sb[:, t * E:(t + 1) * E], tg[:])
            xts_pool_cm.__exit__(None, None, None)

            # -------- local shard routing (softmax/argmax) pre-AllGather,
            # AllGather ships one packed fp32 per token: v = eidx + gv/2.
            # eidx in [0,8), gv in (1/8, 1): frac(v) = gv/2 in (1/16, 1/2),
            # so int-copy recovers eidx under truncation AND round-nearest.
            Lg3 = Lg_sb[:].rearrange("p (t e) -> p t e", e=E)
            lmax_l = rt.tile([128, NTS], f32)
            nc.vector.tensor_reduce(lmax_l[:], Lg3, axis=AX.X, op=OP.max)
            lmax_lb = lmax_l[:].rearrange("p (t o) -> p t o", o=1) \
                               .to_broadcast([128, NTS, E])
            dd_l = rt.tile([128, NTS * E], f32)
            nc.vector.tensor_tensor(out=r3(dd_l[:]), in0=Lg3, in1=lmax_lb,
                                    op=OP.subtract)
            expd_l = rt.tile([128, NTS * E], f32)
            nc.scalar.activation(expd_l[:], dd_l[:], ACTF.Exp)
            ssum_l = rt.tile([128, NTS], f32)
            nc.vector.tensor_reduce(ssum_l[:], r3(expd_l[:]), axis=AX.X,
                                    op=OP.add)
            oh_l = rt.tile([128, NTS * E], f32)
            nc.vector.tensor_tensor(out=r3(oh_l[:]), in0=Lg3, in1=lmax_lb,
                                    op=OP.is_equal)
            tmp_l = rt.tile([128, NTS * E], f32)
            nc.vector.tensor_mul(tmp_l[:], oh_l[:], iota_e[:, :NTS * E])
            v_sb = rt.tile([128, NTS], f32)
            nc.vector.tensor_reduce(v_sb[:], r3(tmp_l[:]), axis=AX.X, op=OP.add)
            gv_l = rt.tile([128, NTS], f32)
            nc.vector.reciprocal(gv_l[:], ssum_l[:])
            nc.vector.tensor_scalar(out=gv_l[:], in0=gv_l[:], scalar1=0.5,
                                    scalar2=None, op0=OP.mult)
            nc.vector.tensor_add(v_sb[:], v_sb[:], gv_l[:])

            # partition-major payload so the post-AG load moves 32B runs
            nc.sync.dma_start(Lg_in[:].rearrange("(p t) -> p t", p=128),
                              v_sb[:])
            nc.gpsimd.collective_compute(
                "AllGather", OP.bypass,
                replica_groups=[list(range(N_CORES))],
                ins=[Lg_in[:]], outs=[Lg_all[:]])
            warm_open()

            # release the w1 stream only after the Lg_in payload DMA has
            # fully landed (readback creates the dep): DMA engine queues are
            # FIFO per engine, so 8 MiB of w1 enqueued earlier would delay
            # the 4 KiB collective payload by ~10us.
            Lg_chk = rt.tile([1, 8], f32)
            nc.sync.dma_start(Lg_chk[0:1, :],
                              Lg_in[0:8].rearrange("(o t) -> o t", o=1))
            nc.vector.tensor_copy(w1_sb[0:1, 0:1], Lg_chk[0:1, 0:1])
            for qq in range(4):
                nc.scalar.dma_start(
                    w1_sb[:].rearrange("p (mc h) -> p mc h", h=H)
                    [:, qq * 2:(qq + 1) * 2, :],
                    w1_ap.rearrange("(mc p) h -> p mc h", p=128)
                    [:, qq * 2:(qq + 1) * 2, :])

            # dummy indirect gather gated on Lg_sb: keeps the dynamic DMA
            # queue spun up so the real gathers don't pay cold-start.
            zoff2 = cp.tile([128, 1], i32)
            nc.vector.tensor_scalar(out=zoff2[:], in0=Lg_sb[:, 0:1],
                                    scalar1=0.0, scalar2=None, op0=OP.mult)
            dumg2 = cp.tile([128, 64], f16)
            nc.gpsimd.indirect_dma_start(
                out=dumg2[:], out_offset=None, in_=x16_ap[:, 0:64],
                in_offset=bass.IndirectOffsetOnAxis(ap=zoff2[:], axis=0))

            bigp_cm = tc.tile_pool(name="big", bufs=1)
            bigp = bigp_cm.__enter__()
            wk_cm = tc.tile_pool(name="wk", bufs=2)
            wk = wk_cm.__enter__()

            # ---------------- phase 2: routing (replicated) ----------------
            # load the packed payload and decode eidx / gv
            L4 = rt.tile([128, NT], f32)
            nc.sync.dma_start(
                L4[:].rearrange("p (c t) -> p c t", t=NTS),
                Lg_all[:].rearrange("(c p t) -> p c t", p=128, t=NTS))
            warm_anchor(L4[:, 0:8], 44)

            ei32 = rt.tile([128, NT], i32)
            nc.vector.tensor_copy(ei32[:], L4[:])
            ef = rt.tile([128, NT], f32)
            nc.vector.tensor_copy(ef[:], ei32[:])
            gv = rt.tile([128, NT], f32)
            nc.vector.tensor_tensor(out=gv[:], in0=L4[:], in1=ef[:],
                                    op=OP.subtract)
            nc.vector.tensor_scalar(out=gv[:], in0=gv[:], scalar1=2.0,
                                    scalar2=None, op0=OP.mult)
            ef_b = ef[:].rearrange("p (t o) -> p t o", o=1) \
                        .to_broadcast([128, NT, E])
            oh = rt.tile([128, NT * E], f32)
            nc.vector.tensor_tensor(out=r3(oh[:]), in0=r3(iota_e[:]), in1=ef_b,
                                    op=OP.is_equal)

            ps_rt_cm = tc.tile_pool(name="ps_rt", bufs=1, space="PSUM")
            ps_rt = ps_rt_cm.__enter__()
            pos_ps = ps_rt.tile([128, NT * E], f32, tag="pos")
            nc.tensor.matmul(pos_ps[:], TRI[:], oh[:], start=True, stop=False)
            # per-(tile,expert) counts in one matmul: ones_col.T @ oh
            cnt1_ps = ps_rt.tile([1, NT * E], f32, tag="misc", name="cnt1")
            nc.tensor.matmul(cnt1_ps[:], ones_col[:], oh[:],
                             start=True, stop=True)
            cnt_sb = rt.tile([1, NT * E], f32)
            nc.vector.tensor_copy(cnt_sb[:], cnt1_ps[:])
            cnt64 = rt.tile([64, 8], f32)
            nc.scalar.dma_start(cnt64[:], cnt_sb[0:1, :])
            offs_ps = ps_rt.tile([64, 8], f32, tag="misc", name="offsps")
            nc.tensor.matmul(offs_ps[:], TRIS[:], cnt64[:], start=True, stop=True)
            offs_sb = rt.tile([64, 8], f32)
            nc.vector.tensor_copy(offs_sb[:], offs_ps[:])
            # direct SBUF->SBUF relayout [64p,8f] -> [1p,512f] (row-major both)
            offs_flat = rt.tile([1, NT * E], f32)
            nc.scalar.dma_start(offs_flat[0:1, :], offs_sb[:])
            nc.tensor.matmul(pos_ps[:], ones_row[:], offs_flat[:],
                             start=False, stop=True)

            pm = rt.tile([128, NT * E], f32)
            nc.vector.tensor_mul(pm[:], pos_ps[:], oh[:])
            pos_tok = rt.tile([128, NT], f32)
            nc.vector.tensor_reduce(pos_tok[:], r3(pm[:]), axis=AX.X, op=OP.add)
            nc.vector.tensor_scalar_add(pos_tok[:], pos_tok[:], -1.0)

            keep = rt.tile([128, NT], f32)
            nc.vector.tensor_scalar(out=keep[:], in0=pos_tok[:],
                                    scalar1=float(C), scalar2=None, op0=OP.is_lt)
            mine = rt.tile([128, NT], f32)
            nc.vector.tensor_scalar(out=mine[:], in0=ef[:],
                                    scalar1=eid_sb[:, 0:1], scalar2=None,
                                    op0=OP.is_equal)
            sel = rt.tile([128, NT], f32)
            nc.vector.tensor_mul(sel[:], mine[:], keep[:])
            gvk = rt.tile([128, NT], f32)
            nc.vector.tensor_mul(gvk[:], gv[:], keep[:])

            # packed payload: tokid*2048 + round(gv*2047) if sel else -1;
            # cols NT..NT+7 stay 0 (always-kept sentinels -> A=0, G=0)
            XCOL = NT + C // 128
            mtok = rt.tile([128, XCOL], f32)
            nc.vector.memset(mtok[:, NT:], 0.0)
            nc.vector.tensor_scalar(out=mtok[:, :NT], in0=tokf[:], scalar1=2048.0,
                                    scalar2=1.0, op0=OP.mult, op1=OP.add)
            gq = rt.tile([128, NT], f32)
            nc.vector.tensor_scalar_mul(gq[:], gvk[:], 2047.0)
            nc.vector.tensor_add(mtok[:, :NT], mtok[:, :NT], gq[:])
            nc.vector.tensor_mul(mtok[:, :NT], mtok[:, :NT], sel[:])
            nc.vector.tensor_scalar_add(mtok[:, :NT], mtok[:, :NT], -1.0)

            # ------ sparse_gather compaction: slot -> (tokid, gate) ------
            # V1 [16, 576] is built entirely in SBUF with two rounds of PE
            # transposes (a DRAM staging hop here measured 4B-packet-bound).
            # V1[q, t*8+u] = mtok[u*16+q, t].
            tps1 = ps_rt.tile([128, 128], f32, tag="vdt")
            nc.tensor.transpose(tps1[0:XCOL, :], mtok[:], IDN[:])
            mtokT = rt.tile([128, 128], f32)
            nc.vector.tensor_copy(mtokT[0:XCOL, :], tps1[0:XCOL, :])
            V1 = rt.tile([16, F_IN], f32)
            V1v = V1[:].rearrange("q (t u) -> q t u", u=8)
            for u in range(8):
                tpsV = ps_rt.tile([16, XCOL], f32, tag="tpsV", name="tpsV")
                nc.tensor.transpose(tpsV[:], mtokT[0:XCOL, u * 16:(u + 1) * 16],
                                    IDN[0:XCOL, 0:XCOL])
                nc.vector.tensor_copy(V1v[:, :, u], tpsV[:])

            # dummy indirect gather gated on mtokT: re-warm the dynamic
            # queue just before the real gathers.
            zoff3 = cp.tile([128, 1], i32)
            nc.vector.tensor_scalar(out=zoff3[:], in0=mtokT[:, 0:1],
                                    scalar1=0.0, scalar2=None, op0=OP.mult)
            dumg3 = cp.tile([128, 64], f16)
            nc.gpsimd.indirect_dma_start(
                out=dumg3[:], out_offset=None, in_=x16_ap[:, 0:64],
                in_offset=bass.IndirectOffsetOnAxis(ap=zoff3[:], axis=0))

            ps_rt_cm.__exit__(None, None, None)
            SG1 = rt.tile([16, F_OUT], f32)
            NF1 = rt.tile([1, 1], u32)
            nc.gpsimd.sparse_gather(SG1[:], V1[:], num_found=NF1[:])

            # The device-side slot order is a fixed permutation of the
            # reference slot order (sparse_gather's native [16,F'] order
            # read back contiguously). A, G and EOT all use the same
            # permutation, and the host aligns them by index, so this is
            # transparent -- and every hop here is contiguous.
            A_fq = dram.tile([C], f32)          # layout: q*64 + f
            nc.scalar.dma_start(A_fq[:].rearrange("(q f) -> q f", q=16), SG1[:])
            p_slot = rt.tile([128, C // 128], f32)
            nc.sync.dma_start(p_slot[:],
                              A_fq[:].rearrange("(p j) -> p j", p=128))
            p_i = rt.tile([128, C // 128], i32)
            nc.vector.tensor_copy(p_i[:], p_slot[:])
            a_i = rt.tile([128, C // 128], i32)
            nc.vector.tensor_scalar(out=a_i[:], in0=p_i[:], scalar1=11,
                                    scalar2=None, op0=OP.arith_shift_right)
            nc.scalar.dma_start(a_ap.rearrange("(s p) o -> p (s o)", p=128), a_i[:])
            gq_i = rt.tile([128, C // 128], i32)
            nc.vector.tensor_scalar(out=gq_i[:], in0=p_i[:], scalar1=2047,
                                    scalar2=None, op0=OP.bitwise_and)
            g_sb = rt.tile([128, C // 128], f32)
            nc.vector.tensor_copy(g_sb[:], gq_i[:])
            nc.vector.tensor_scalar_mul(g_sb[:], g_sb[:], 1.0 / 2047.0)
            nc.scalar.dma_start(g_ap.rearrange("(s p) o -> p (s o)", p=128), g_sb[:])
            G_f = dram.tile([C], f32)
            nc.scalar.dma_start(G_f[:].rearrange("(s p) -> p s", p=128), g_sb[:])

            # ---------------- phase 3: dispatch gather + transpose --------
            # gathers from fp16 x copy; PE transposes in fp16 (single pass)
            # g_sb-anchored warm burst: ~3.5us of PE busy right before the
            # transposes so they and GEMM1 start at full clock.
            warm_anchor(g_sb[:, 0:8], 56)
            dispTs = [bigp.tile([128, (M // 128) * CBLK], f16, name=f"dispT{c}")
                      for c in range(NCB)]  # [p, (mc, c_in_blk)]
            warm_close()
            ps_tr_cm = tc.tile_pool(name="ps_tr", bufs=2, space="PSUM")
            ps_tr = ps_tr_cm.__enter__()
            dgs = []

            def gather_sg(sg):
                dg = wk.tile([128, M], f16, tag="dg", bufs=3)
                nc.gpsimd.indirect_dma_start(
                    out=dg[:], out_offset=None, in_=x16_ap,
                    in_offset=bass.IndirectOffsetOnAxis(
                        ap=a_i[:, sg:sg + 1], axis=0))
                return dg

            def transpose_sg(sg, dg):
                cbb, sgo = divmod(sg, CBLK // 128)
                for mc in range(M // 128):
                    tp = ps_tr.tile([128, 128], f16, tag="tr")
                    nc.tensor.transpose(tp[:], dg[:, mc * 128:(mc + 1) * 128],
                                        IDN16[:])
                    nc.vector.tensor_copy(
                        dispTs[cbb][:, mc * CBLK + sgo * 128:
                                    mc * CBLK + (sgo + 1) * 128],
                        tp[:])

            # issue all gathers; transpose first c-block's 4 groups now,
            # the rest are interleaved into GEMM1 below so the tensor queue
            # stays available.
            for sg in range(C // 128):
                dgs.append(gather_sg(sg))
            for sg in range(4):
                transpose_sg(sg, dgs[sg])

            # ---------------- phases 4+5: expert FFN ----------------
            ps_ffn_cm = tc.tile_pool(name="ps_ffn", bufs=1, space="PSUM")
            ps_ffn = ps_ffn_cm.__enter__()
            hTs = [bigp.tile([128, (H // 128) * CBLK], f16, name=f"hT{c}")
                   for c in range(NCB)]
            w1v = w1_sb[:].rearrange("p (mc h) -> p mc h", h=H)
            for cb in range(NCB):
                hT = hTs[cb]
                # GEMM1: hT[h, c] = relu(w1.T @ dispT + b1)
                for htb in range(H // CBLK):              # 8 blocks of 4 ht
                    for hi in range(CBLK // 128):         # 4 ht per block
                        ht_i = htb * (CBLK // 128) + hi
                        ps1 = ps_ffn.tile([128, CBLK], f32, tag="g1", bufs=4)
                        for mc in range(M // 128):
                            nc.tensor.matmul(
                                ps1[:],
                                w1v[:, mc,
                                    htb * CBLK + hi * 128:
                                    htb * CBLK + (hi + 1) * 128],
                                dispTs[cb][:, mc * CBLK:(mc + 1) * CBLK],
                                start=(mc == 0), stop=(mc == M // 128 - 1))
                        nc.scalar.activation(
                            hT[:, ht_i * CBLK:(ht_i + 1) * CBLK], ps1[:],
                            ACTF.Relu, bias=b1c[:, ht_i:ht_i + 1], scale=1.0)
                    if cb == 0 and htb < 4:
                        # second c-block's transposes, interleaved
                        transpose_sg(4 + htb, dgs[4 + htb])

                # gate row broadcast for this c-block
                g_row = wk.tile([1, CBLK], f32, tag="grow")
                nc.sync.dma_start(g_row[0:1, :],
                                  G_f[cb * CBLK:(cb + 1) * CBLK])
                gb_ps = ps_ffn.tile([128, CBLK], f32, tag="g1", bufs=4)
                nc.tensor.matmul(gb_ps[:], ones_row[:], g_row[:],
                                 start=True, stop=True)
                g_bc = wk.tile([128, CBLK], f32, tag="gbc")
                nc.vector.tensor_copy(g_bc[:], gb_ps[:])

                # GEMM2: eoT[m, c] = w2.T @ hT ; then (+b2) * gate
                for mt in range(M // 128):
                    w2ts = wk.tile([128, H], f16, tag="w2ts", bufs=2)
                    nc.scalar.dma_start(
                        w2ts[:], w2t_ap[mt].rearrange("p (hc m) -> p hc m", m=128))
                    ps2 = ps_ffn.tile([128, CBLK], f32, tag="g2", bufs=2)
                    for hc in range(H // 128):
                        nc.tensor.matmul(
                            ps2[:], w2ts[:, hc * 128:(hc + 1) * 128],
                            hT[:, hc * CBLK:(hc + 1) * CBLK],
                            start=(hc == 0), stop=(hc == H // 128 - 1))
                    eo_sb = wk.tile([128, CBLK], f16, tag="eo")
                    nc.vector.tensor_scalar(out=eo_sb[:], in0=ps2[:],
                                            scalar1=b2c[:, mt:mt + 1],
                                            scalar2=None, op0=OP.add)
                    nc.vector.tensor_mul(eo_sb[:], eo_sb[:], g_bc[:])
                    nc.sync.dma_start(
                        eot_ap[mt * 128:(mt + 1) * 128,
                               cb * CBLK:(cb + 1) * CBLK], eo_sb[:])

            ps_ffn_cm.__exit__(None, None, None)
            ps_tr_cm.__exit__(None, None, None)
            wk_cm.__exit__(None, None, None)
            bigp_cm.__exit__(None, None, None)

    nc.compile()
    _split_multi_waits(nc)
    return nc


_NC_CACHE = None


def _get_nc():
    global _NC_CACHE
    if _NC_CACHE is None:
        _NC_CACHE = build()
    return _NC_CACHE


def _make_in_maps(x, wg, w1, b1, w2, b2):
    x2 = np.ascontiguousarray(np.asarray(x, np.float32).reshape(S, M))
    x16 = np.ascontiguousarray(x2.astype(np.float16))
    wg = np.ascontiguousarray(np.asarray(wg, np.float32))
    w1 = np.asarray(w1, np.float32)
    b1 = np.asarray(b1, np.float32)
    w2 = np.asarray(w2, np.float32)
    b2 = np.asarray(b2, np.float32)
    in_maps = []
    for k in range(N_CORES):
        shard = x2[k * (S // N_CORES):(k + 1) * (S // N_CORES)]
        xts = np.ascontiguousarray(shard.T)                    # [M, S/8]
        w1k = np.ascontiguousarray(w1[k]).astype(np.float16)   # [M, H]
        b1ck = np.ascontiguousarray(b1[k].reshape(H // 128, 128).T)
        w2k = w2[k]                                            # [H, M]
        w2t = np.ascontiguousarray(
            w2k.reshape(H // 128, 128, M // 128, 128).transpose(2, 1, 0, 3)
        ).astype(np.float16)
        b2ck = np.ascontiguousarray(b2[k].reshape(M // 128, 128).T)
        eid = np.full((128, 1), k, np.float32)
        in_maps.append({
            "x16": x16, "xts": xts, "wg": wg, "w1": w1k, "b1c": b1ck,
            "w2t": w2t, "b2c": b2ck, "eid": eid,
        })
    return in_maps


def run_cores(x, wg, w1, b1, w2, b2, trace=False, tmpdir=None):
    nc = _get_nc()
    in_maps = _make_in_maps(x, wg, w1, b1, w2, b2)
    return run_bass_kernel_spmd(nc, in_maps, list(range(N_CORES)), trace=trace,
                                tmpdir=tmpdir)


def combine(results):
    out = np.zeros((S, M), np.float32)
    for k in range(N_CORES):
        r = results[k]
        eo = np.ascontiguousarray(r["EOT"].astype(np.float32).T)  # [C, M]
        A = r["A"][:, 0].astype(np.int64)
        G = r["G"][:, 0]
        valid = G > 0
        out[A[valid]] = eo[valid]
    return out.reshape(B, SQ, M)


def kernel(x, wg, w1, b1, w2, b2):
    res = run_cores(x, wg, w1, b1, w2, b2, trace=False)
    return combine(res.results)


# revision 37
# speedup vs baseline: 1.1578x; 1.0189x over previous
"""MoE (top-1, capacity_factor=1) Trainium2 Bass kernel, expert-parallel over
8 NeuronCores. Self-contained: imports only numpy + concourse (/opt).

Per-core k (expert k resident):
  gate GEMM (fp32 exact) on its 1/8 token shard -> AllGather logits ->
  replicated routing (one-hot, global cumsum via triangular matmuls,
  capacity mask) -> slot->token table via gpsimd sparse_gather ->
  indirect row-gather of x (fp16) + PE transposes -> dispT [m, c] ->
  GEMM1 (fp16, w1 resident in SBUF) -> hT [h, c] with fused ReLU+b1 ->
  GEMM2 (fp16, w2 streamed) -> eoT [m, c], + b2 + gate scaling -> EOT (f16).
Host: scatter rows eo = EOT.T back by token id (A table), empties G==0.

v2: PE kept warm from t=0; dummy collective to absorb CC-stack warmup;
SBUF->SBUF relayouts instead of DRAM round-trips where legal; strided
loads split across sync/scalar queues; x gathered as fp16 (half bytes);
w1 fully resident; EOT in fp16.
"""
import sys

sys.path.insert(0, '/opt/trn_rl_repo')

import numpy as np
import concourse.bass as bass
import concourse.tile as tile
import concourse.mybir as mybir
from concourse import bacc
from concourse.bass_utils import run_bass_kernel_spmd
from concourse.masks import make_upper_triangular, make_identity

N_CORES = 8
B, SQ, M, E, H = 4, 2048, 1024, 8, 4096
S = B * SQ            # 8192 tokens
C = S // E            # 1024 capacity
NT = S // 128         # 64 token tiles
NTS = NT // N_CORES   # 8 token tiles per core shard
CBLK = 512            # c-block for GEMM1/GEMM2
NCB = C // CBLK       # 2 c-blocks
F_IN = (S + C) // 16  # 576  sparse_gather input free size
F_OUT = C // 16       # 64

f32 = mybir.dt.float32
f16 = mybir.dt.float16
bf16 = mybir.dt.bfloat16
i32 = mybir.dt.int32
u32 = mybir.dt.uint32
AX = mybir.AxisListType
OP = mybir.AluOpType
ACTF = mybir.ActivationFunctionType


def _split_multi_waits(nc):
    """This walrus build accepts at most ONE sync-wait per instruction.
    Split extras into same-engine NOPs inserted just before."""
    from concourse.mybir import SyncInfo
    n = 0
    for bb in list(nc.main_func.blocks):
        insts = bb.instructions  # live shared list
        for ins in list(insts):
            si = ins.sync_info
            if si is None or len(si.on_wait) <= 1:
                continue
            waits = list(si.on_wait)
            idx = insts.index(ins)
            for j, w in enumerate(waits[:-1]):
                nop = nc.engines[ins.engine].nop(nofuse=True, hint="waitsplit")
                ni = nop.ins
                cur = nc.cur_bb.bb.instructions
                if ni in cur:
                    cur.remove(ni)
                ni.sync_info = SyncInfo(on_wait=[w], on_update=[])
                insts.insert(idx + j, ni)
                n += 1
            ins.sync_info = SyncInfo(on_wait=[waits[-1]], on_update=si.on_update)
    return n


def r3(ap, e=E):
    return ap.rearrange("p (t e) -> p t e", e=e)


def build():
    nc = bacc.Bacc("TRN2", target_bir_lowering=False, debug=False,
                   num_devices=N_CORES)

    x16_ap = nc.dram_tensor("x16", [S, M], f16, kind="ExternalInput").ap()
    xts_ap = nc.dram_tensor("xts", [M, S // N_CORES], f32, kind="ExternalInput").ap()
    wg_ap = nc.dram_tensor("wg", [M, E], f32, kind="ExternalInput").ap()
    w1_ap = nc.dram_tensor("w1", [M, H], f16, kind="ExternalInput").ap()
    b1c_ap = nc.dram_tensor("b1c", [128, H // 128], f32, kind="ExternalInput").ap()
    w2t_ap = nc.dram_tensor("w2t", [M // 128, 128, H], f16, kind="ExternalInput").ap()
    b2c_ap = nc.dram_tensor("b2c", [128, M // 128], f32, kind="ExternalInput").ap()
    eid_ap = nc.dram_tensor("eid", [128, 1], f32, kind="ExternalInput").ap()

    eot_ap = nc.dram_tensor("EOT", [M, C], f16, kind="ExternalOutput").ap()
    a_ap = nc.dram_tensor("A", [C, 1], i32, kind="ExternalOutput").ap()
    g_ap = nc.dram_tensor("G", [C, 1], f32, kind="ExternalOutput").ap()

    with tile.TileContext(nc) as tc:
        with tc.tile_pool(name="consts", bufs=1) as cp, \
             tc.tile_pool(name="rt", bufs=1) as rt, \
             tc.tile_pool(name="dram", bufs=1, space="DRAM") as dram:

            # gate inputs first: these DMAs gate the whole pipeline.
            # xts strips split across both HWDGE trigger queues.
            wg_sb = cp.tile([128, (M // 128) * E], f32)   # [p, (mc, e)]
            nc.sync.dma_start(wg_sb[:].rearrange("p (t e) -> p t e", e=E),
                              wg_ap.rearrange("(mc p) e -> p mc e", p=128))
            xts_pool_cm = tc.tile_pool(name="xts", bufs=1)
            xp = xts_pool_cm.__enter__()
            xts_strips = []
            for mc in range(M // 128):
                st = xp.tile([128, S // N_CORES], f32, tag=f"xts{mc}",
                             name=f"xts{mc}")
                eng = nc.sync if mc % 2 == 0 else nc.scalar
                eng.dma_start(st[:], xts_ap[mc * 128:(mc + 1) * 128, :])
                xts_strips.append(st)

            # w1 fully resident in SBUF. The DMAs are gated behind the gate
            # GEMM output (WAW on w1_sb via the seed copy below) so the 8 MiB
            # stream does not compete with the latency-critical xts load.
            w1_sb = cp.tile([128, (M // 128) * H], f16)   # [p, (mc, h)]

            # ---------------- constants ----------------
            TRI = cp.tile([128, 128], f32)       # TRI[q,p]=1 iff q<=p
            make_upper_triangular(nc, TRI[:], val=1.0, diag=True)
            TRIS = cp.tile([64, 64], f32)        # strict upper
            make_upper_triangular(nc, TRIS[:], val=1.0, diag=False)
            IDN = cp.tile([128, 128], f32)
            make_identity(nc, IDN[:])
            IDN16 = cp.tile([128, 128], f16)
            make_identity(nc, IDN16[:])
            ones_col = cp.tile([128, 1], f32)
            nc.vector.memset(ones_col[:], 1.0)
            ones_row = cp.tile([1, 128], f32)
            nc.vector.memset(ones_row[:], 1.0)
            iota_e = cp.tile([128, NT * E], f32)
            nc.gpsimd.iota(r3(iota_e[:]), pattern=[[0, NT], [1, E]], base=0,
                           channel_multiplier=0,
                           allow_small_or_imprecise_dtypes=True)
            tokf = cp.tile([128, NT], f32)
            nc.gpsimd.iota(tokf[:], pattern=[[128, NT]], base=0,
                           channel_multiplier=1,
                           allow_small_or_imprecise_dtypes=True)
            eid_sb = cp.tile([128, 1], f32)
            nc.scalar.dma_start(eid_sb[:], eid_ap[:])
            b1c = cp.tile([128, H // 128], f32)
            nc.scalar.dma_start(b1c[:], b1c_ap[:])
            b2c = cp.tile([128, M // 128], f32)
            nc.scalar.dma_start(b2c[:], b2c_ap[:])

            # preload the sparse_gather ucode library early
            dumv = cp.tile([16, 8], f32)
            nc.vector.memset(dumv[:], 1.0)
            dumo = cp.tile([16, 8], f32)
            dumn = cp.tile([1, 1], u32)
            nc.gpsimd.sparse_gather(dumo[:], dumv[:], num_found=dumn[:])
            zoff = cp.tile([128, 1], i32)
            nc.vector.memset(zoff[:], 0)
            dumg = cp.tile([128, 64], f16)
            nc.gpsimd.indirect_dma_start(
                out=dumg[:], out_offset=None, in_=x16_ap[:, 0:64],
                in_offset=bass.IndirectOffsetOnAxis(ap=zoff[:], axis=0))

            # PE warm machinery: keep the HAM activity monitor fed so real
            # matmuls run at full clock. PSUM banks are pool-granular, so the
            # warm pool opens/closes around phases that need all 8 banks.
            warm_in = cp.tile([128, 64], bf16)
            nc.vector.memset(warm_in[:], 1.0)
            warm_w = cp.tile([128, 64], bf16)
            nc.vector.memset(warm_w[:], 1.0)
            warm_state = {}

            def warm_open(nt=4):
                cm = tc.tile_pool(name="warm_ps", bufs=1, space="PSUM")
                pool = cm.__enter__()
                warm_state['cm'] = cm
                # round-robin output tiles: WAW deps on a single tile
                # would serialize the warm matmuls into isolated (drain-
                # latency) issues and clog the tensor queue.
                warm_state['outs'] = [
                    pool.tile([64, 64], f32, tag=f"warm{j}", name=f"warm{j}")
                    for j in range(nt)]
                warm_state['rr'] = 0

            def warm_close():
                warm_state.pop('cm').__exit__(None, None, None)

            def pe_warm(n):
                k = len(warm_state['outs'])
                for _ in range(n):
                    j = warm_state['rr'] = (warm_state['rr'] + 1) % k
                    nc.tensor.matmul(warm_state['outs'][j][:], warm_w[:],
                                     warm_in[:], start=True, stop=True,
                                     skip_group_check=True)

            warm_in32 = cp.tile([128, 64], f32)
            nc.vector.memset(warm_in32[:], 1.0)

            def warm_anchor(src_ap, np_):
                # a matmul that READS src_ap chains the warm burst behind
                # that tile's producer (the scheduler reorders freely, so
                # data deps are the only reliable sequencing tool).
                nc.tensor.matmul(warm_state['outs'][0][0:src_ap.shape[1], 0:64],
                                 src_ap, warm_in32[:, 0:64],
                                 start=True, stop=True, skip_group_check=True)
                pe_warm(np_)

            # ~4us of initial PE busy to trip the HAM SHORT window before
            # the gate GEMM begins, plus bursts anchored on the gate's
            # inputs so the PE stays warm until the first strip lands.
            warm_open()
            pe_warm(64)
            warm_anchor(wg_sb[:, 0:8], 8)
            warm_anchor(xts_strips[0][:, 0:8], 16)
            warm_close()

            # ---------------- phase 1: gate GEMM (fp32) ----------------
            Lg_sb = rt.tile([128, NTS * E], f32)          # own shard logits
            Lg_in = dram.tile([S // N_CORES], f32)        # packed eidx+gv/2
            Lg_all = dram.tile([S], f32, addr_space="Shared")

            # wg stationary (8-col LDWEIGHTS), tokens streaming N=512: the
            # PE runs at full duty so HAM stays warm; the original
            # token-partition orientation reloaded a 128-col stationary per
            # N=8 matmul and was LDWEIGHTS-latency-bound (~23us cold).
            with tc.tile_pool(name="psg_p", bufs=1, space="PSUM") as psg_p:
                pgs = [psg_p.tile([8, 512], f32, tag=f"pg{h}", name=f"pg{h}")
                       for h in range(2)]
                for mc in range(M // 128):
                    st = xts_strips[mc]
                    for h in range(2):
                        nc.tensor.matmul(
                            pgs[h][:], wg_sb[:, mc * E:(mc + 1) * E],
                            st[:, h * 512:(h + 1) * 512],
                            start=(mc == 0), stop=(mc == M // 128 - 1))
                Lg8 = rt.tile([8, S // N_CORES], f32)
                for h in range(2):
                    nc.vector.tensor_copy(Lg8[:, h * 512:(h + 1) * 512],
                                          pgs[h][:])
                # transpose back to [token-partition, (tile, e)]
                for t in range(NTS):
                    tg = psg_p.tile([128, E], f32, tag="tg", bufs=2, name="tg")
                    nc.tensor.transpose(tg[:], Lg8[:, t * 128:(t + 1) * 128],
                                        IDN[0:8, 0:8])
                    nc.vector.tensor_copy(Lg_sb[:, t * E:(t + 1) * E], tg[:])
            xts_pool_cm.__exit__(None, None, None)

            # -------- local shard routing (softmax/argmax) pre-AllGather,
            # AllGather ships one packed fp32 per token: v = eidx + gv/2.
            # eidx in [0,8), gv in (1/8, 1): frac(v) = gv/2 in (1/16, 1/2),
            # so int-copy recovers eidx under truncation AND round-nearest.
            Lg3 = Lg_sb[:].rearrange("p (t e) -> p t e", e=E)
            lmax_l = rt.tile([128, NTS], f32)
            nc.vector.tensor_reduce(lmax_l[:], Lg3, axis=AX.X, op=OP.max)
            lmax_lb = lmax_l[:].rearrange("p (t o) -> p t o", o=1) \
                               .to_broadcast([128, NTS, E])
            dd_l = rt.tile([128, NTS * E], f32)
            nc.vector.tensor_tensor(out=r3(dd_l[:]), in0=Lg3, in1=lmax_lb,
                                    op=OP.subtract)
            expd_l = rt.tile([128, NTS * E], f32)
            nc.scalar.activation(expd_l[:], dd_l[:], ACTF.Exp)
            ssum_l = rt.tile([128, NTS], f32)
            nc.vector.tensor_reduce(ssum_l[:], r3(expd_l[:]), axis=AX.X,
                                    op=OP.add)
            oh_l = rt.tile([128, NTS * E], f32)
            nc.vector.tensor_tensor(out=r3(oh_l[:]), in0=Lg3, in1=lmax_lb,
                                    op=OP.is_equal)
            tmp_l = rt.tile([128, NTS * E], f32)
            nc.vector.tensor_mul(tmp_l[:], oh_l[:], iota_e[:, :NTS * E])
            v_sb = rt.tile([128, NTS], f32)
            nc.vector.tensor_reduce(v_sb[:], r3(tmp_l[:]), axis=AX.X, op=OP.add)
            gv_l = rt.tile([128, NTS], f32)
            nc.vector.reciprocal(gv_l[:], ssum_l[:])
            nc.vector.tensor_scalar(out=gv_l[:], in0=gv_l[:], scalar1=0.5,
                                    scalar2=None, op0=OP.mult)
            nc.vector.tensor_add(v_sb[:], v_sb[:], gv_l[:])

            # partition-major payload so the post-AG load moves 32B runs;
            # split across both HWDGE queues to halve completion latency
            Lg_inv = Lg_in[:].rearrange("(p t) -> p t", p=128)
            nc.sync.dma_start(Lg_inv[:, 0:4], v_sb[:, 0:4])
            nc.scalar.dma_start(Lg_inv[:, 4:8], v_sb[:, 4:8])
            nc.gpsimd.collective_compute(
                "AllGather", OP.bypass,
                replica_groups=[list(range(N_CORES))],
                ins=[Lg_in[:]], outs=[Lg_all[:]])
            warm_open(nt=2)

            # release the w1 stream only after the Lg_in payload DMA has
            # fully landed (readback creates the dep): DMA engine queues are
            # FIFO per engine, so 8 MiB of w1 enqueued earlier would delay
            # the 4 KiB collective payload by ~10us.
            Lg_chk = rt.tile([1, 8], f32)
            nc.sync.dma_start(Lg_chk[0:1, :],
                              Lg_in[0:8].rearrange("(o t) -> o t", o=1))
            nc.vector.tensor_copy(w1_sb[0:1, 0:1], Lg_chk[0:1, 0:1])
            for qq in range(4):
                nc.scalar.dma_start(
                    w1_sb[:].rearrange("p (mc h) -> p mc h", h=H)
                    [:, qq * 2:(qq + 1) * 2, :],
                    w1_ap.rearrange("(mc p) h -> p mc h", p=128)
                    [:, qq * 2:(qq + 1) * 2, :])

            # dummy indirect gather gated on Lg_sb: keeps the dynamic DMA
            # queue spun up so the real gathers don't pay cold-start.
            zoff2 = cp.tile([128, 1], i32)
            nc.vector.tensor_scalar(out=zoff2[:], in0=Lg_sb[:, 0:1],
                                    scalar1=0.0, scalar2=None, op0=OP.mult)
            dumg2 = cp.tile([128, 64], f16)
            nc.gpsimd.indirect_dma_start(
                out=dumg2[:], out_offset=None, in_=x16_ap[:, 0:64],
                in_offset=bass.IndirectOffsetOnAxis(ap=zoff2[:], axis=0))

            bigp_cm = tc.tile_pool(name="big", bufs=1)
            bigp = bigp_cm.__enter__()
            wk_cm = tc.tile_pool(name="wk", bufs=2)
            wk = wk_cm.__enter__()

            # ---------------- phase 2: routing (replicated) ----------------
            # load the packed payload and decode eidx / gv
            L4 = rt.tile([128, NT], f32)
            L4v = L4[:].rearrange("p (c t) -> p c t", t=NTS)
            Lg_allv = Lg_all[:].rearrange("(c p t) -> p c t", p=128, t=NTS)
            nc.sync.dma_start(L4v[:, 0:4, :], Lg_allv[:, 0:4, :])
            nc.scalar.dma_start(L4v[:, 4:8, :], Lg_allv[:, 4:8, :])
            warm_anchor(L4[:, 0:8], 48)

            # chained dummy gathers: keep the dynamic DMA path streaming
            # through the routing phase so the real gathers don't pay a
            # cold-start penalty.
            zoff4 = cp.tile([128, 1], i32)
            nc.vector.tensor_scalar(out=zoff4[:], in0=L4[:, 0:1],
                                    scalar1=0.0, scalar2=None, op0=OP.mult)
            dumg4 = cp.tile([128, 64], f16)
            for _ in range(6):
                nc.gpsimd.indirect_dma_start(
                    out=dumg4[:], out_offset=None, in_=x16_ap[:, 0:64],
                    in_offset=bass.IndirectOffsetOnAxis(ap=zoff4[:], axis=0))

            ei32 = rt.tile([128, NT], i32)
            nc.vector.tensor_copy(ei32[:], L4[:])
            ef = rt.tile([128, NT], f32)
            nc.vector.tensor_copy(ef[:], ei32[:])
            gv = rt.tile([128, NT], f32)
            nc.vector.tensor_tensor(out=gv[:], in0=L4[:], in1=ef[:],
                                    op=OP.subtract)
            nc.vector.tensor_scalar(out=gv[:], in0=gv[:], scalar1=2.0,
                                    scalar2=None, op0=OP.mult)
            ef_b = ef[:].rearrange("p (t o) -> p t o", o=1) \
                        .to_broadcast([128, NT, E])
            oh = rt.tile([128, NT * E], f32)
            nc.vector.tensor_tensor(out=r3(oh[:]), in0=r3(iota_e[:]), in1=ef_b,
                                    op=OP.is_equal)

            ps_rt_cm = tc.tile_pool(name="ps_rt", bufs=1, space="PSUM")
            ps_rt = ps_rt_cm.__enter__()
            pos_ps = ps_rt.tile([128, NT * E], f32, tag="pos")
            nc.tensor.matmul(pos_ps[:], TRI[:], oh[:], start=True, stop=False)
            # per-(tile,expert) counts straight into [64,8] partitions
            cnt_ps = ps_rt.tile([64, 8], f32, tag="misc", name="cntps")
            oh3 = oh[:].rearrange("p (t e) -> p e t", e=E)
            for e in range(E):
                nc.tensor.matmul(cnt_ps[:, e:e + 1], oh3[:, e, :], ones_col[:],
                                 start=True, stop=True)
            cnt64 = rt.tile([64, 8], f32)
            nc.vector.tensor_copy(cnt64[:], cnt_ps[:])
            offs_ps = ps_rt.tile([64, 8], f32, tag="misc", name="offsps")
            nc.tensor.matmul(offs_ps[:], TRIS[:], cnt64[:], start=True, stop=True)
            offs_sb = rt.tile([64, 8], f32)
            nc.vector.tensor_copy(offs_sb[:], offs_ps[:])
            # direct SBUF->SBUF relayout [64p,8f] -> [1p,512f] (row-major both)
            offs_flat = rt.tile([1, NT * E], f32)
            nc.scalar.dma_start(offs_flat[0:1, :], offs_sb[:])
            nc.tensor.matmul(pos_ps[:], ones_row[:], offs_flat[:],
                             start=False, stop=True)

            pm = rt.tile([128, NT * E], f32)
            nc.vector.tensor_mul(pm[:], pos_ps[:], oh[:])
            pos_tok = rt.tile([128, NT], f32)
            nc.vector.tensor_reduce(pos_tok[:], r3(pm[:]), axis=AX.X, op=OP.add)
            nc.vector.tensor_scalar_add(pos_tok[:], pos_tok[:], -1.0)

            keep = rt.tile([128, NT], f32)
            nc.vector.tensor_scalar(out=keep[:], in0=pos_tok[:],
                                    scalar1=float(C), scalar2=None, op0=OP.is_lt)
            mine = rt.tile([128, NT], f32)
            nc.vector.tensor_scalar(out=mine[:], in0=ef[:],
                                    scalar1=eid_sb[:, 0:1], scalar2=None,
                                    op0=OP.is_equal)
            sel = rt.tile([128, NT], f32)
            nc.vector.tensor_mul(sel[:], mine[:], keep[:])
            gvk = rt.tile([128, NT], f32)
            nc.vector.tensor_mul(gvk[:], gv[:], keep[:])

            # packed payload: tokid*2048 + round(gv*2047) if sel else -1;
            # cols NT..NT+7 stay 0 (always-kept sentinels -> A=0, G=0)
            XCOL = NT + C // 128
            mtok = rt.tile([128, XCOL], f32)
            nc.vector.memset(mtok[:, NT:], 0.0)
            nc.vector.tensor_scalar(out=mtok[:, :NT], in0=tokf[:], scalar1=2048.0,
                                    scalar2=1.0, op0=OP.mult, op1=OP.add)
            gq = rt.tile([128, NT], f32)
            nc.vector.tensor_scalar_mul(gq[:], gvk[:], 2047.0)
            nc.vector.tensor_add(mtok[:, :NT], mtok[:, :NT], gq[:])
            nc.vector.tensor_mul(mtok[:, :NT], mtok[:, :NT], sel[:])
            nc.vector.tensor_scalar_add(mtok[:, :NT], mtok[:, :NT], -1.0)

            # ------ sparse_gather compaction: slot -> (tokid, gate) ------
            # V1 [16, 576] is built entirely in SBUF with two rounds of PE
            # transposes (a DRAM staging hop here measured 4B-packet-bound).
            # V1[q, t*8+u] = mtok[u*16+q, t].
            tps1 = ps_rt.tile([128, 128], f32, tag="vdt")
            nc.tensor.transpose(tps1[0:XCOL, :], mtok[:], IDN[:])
            mtokT = rt.tile([128, 128], f32)
            nc.vector.tensor_copy(mtokT[0:XCOL, :], tps1[0:XCOL, :])
            V1 = rt.tile([16, F_IN], f32)
            V1v = V1[:].rearrange("q (t u) -> q t u", u=8)
            for u in range(8):
                tpsV = ps_rt.tile([16, XCOL], f32, tag="tpsV", name="tpsV",
                                  bufs=2)
                nc.tensor.transpose(tpsV[:], mtokT[0:XCOL, u * 16:(u + 1) * 16],
                                    IDN[0:XCOL, 0:XCOL])
                nc.vector.tensor_copy(V1v[:, :, u], tpsV[:])

            # dummy indirect gather gated on mtokT: re-warm the dynamic
            # queue just before the real gathers.
            zoff3 = cp.tile([128, 1], i32)
            nc.vector.tensor_scalar(out=zoff3[:], in0=mtokT[:, 0:1],
                                    scalar1=0.0, scalar2=None, op0=OP.mult)
            dumg3 = cp.tile([128, 64], f16)
            nc.gpsimd.indirect_dma_start(
                out=dumg3[:], out_offset=None, in_=x16_ap[:, 0:64],
                in_offset=bass.IndirectOffsetOnAxis(ap=zoff3[:], axis=0))

            ps_rt_cm.__exit__(None, None, None)
            SG1 = rt.tile([16, F_OUT], f32)
            NF1 = rt.tile([1, 1], u32)
            nc.gpsimd.sparse_gather(SG1[:], V1[:], num_found=NF1[:])

            # The device-side slot order is a fixed permutation of the
            # reference slot order (sparse_gather's native [16,F'] order
            # read back contiguously). A, G and EOT all use the same
            # permutation, and the host aligns them by index, so this is
            # transparent -- and every hop here is contiguous.
            A_fq = dram.tile([C], f32)          # layout: q*64 + f
            nc.scalar.dma_start(A_fq[:].rearrange("(q f) -> q f", q=16), SG1[:])
            p_slot = rt.tile([128, C // 128], f32)
            nc.sync.dma_start(p_slot[:],
                              A_fq[:].rearrange("(p j) -> p j", p=128))
            p_i = rt.tile([128, C // 128], i32)
            nc.vector.tensor_copy(p_i[:], p_slot[:])
            a_i = rt.tile([128, C // 128], i32)
            nc.vector.tensor_scalar(out=a_i[:], in0=p_i[:], scalar1=11,
                                    scalar2=None, op0=OP.arith_shift_right)
            nc.scalar.dma_start(a_ap.rearrange("(s p) o -> p (s o)", p=128), a_i[:])
            gq_i = rt.tile([128, C // 128], i32)
            nc.vector.tensor_scalar(out=gq_i[:], in0=p_i[:], scalar1=2047,
                                    scalar2=None, op0=OP.bitwise_and)
            g_sb = rt.tile([128, C // 128], f32)
            nc.vector.tensor_copy(g_sb[:], gq_i[:])
            nc.vector.tensor_scalar_mul(g_sb[:], g_sb[:], 1.0 / 2047.0)
            nc.scalar.dma_start(g_ap.rearrange("(s p) o -> p (s o)", p=128), g_sb[:])
            G_f = dram.tile([C], f32)
            nc.scalar.dma_start(G_f[:].rearrange("(s p) -> p s", p=128), g_sb[:])

            # ---------------- phase 3: dispatch gather + transpose --------
            # gathers from fp16 x copy; PE transposes in fp16 (single pass)
            # g_sb-anchored warm burst: ~3.5us of PE busy right before the
            # transposes so they and GEMM1 start at full clock.
            warm_anchor(g_sb[:, 0:8], 56)
            dispTs = [bigp.tile([128, (M // 128) * CBLK], f16, name=f"dispT{c}")
                      for c in range(NCB)]  # [p, (mc, c_in_blk)]
            warm_close()
            ps_tr_cm = tc.tile_pool(name="ps_tr", bufs=2, space="PSUM")
            ps_tr = ps_tr_cm.__enter__()
            dgs = []

            def gather_sg(sg):
                dg = wk.tile([128, M], f16, tag="dg", bufs=4)
                nc.gpsimd.indirect_dma_start(
                    out=dg[:], out_offset=None, in_=x16_ap,
                    in_offset=bass.IndirectOffsetOnAxis(
                        ap=a_i[:, sg:sg + 1], axis=0))
                return dg

            def transpose_sg(sg, dg):
                cbb, sgo = divmod(sg, CBLK // 128)
                for mc in range(M // 128):
                    tp = ps_tr.tile([128, 128], f16, tag="tr")
                    nc.tensor.transpose(tp[:], dg[:, mc * 128:(mc + 1) * 128],
                                        IDN16[:])
                    nc.vector.tensor_copy(
                        dispTs[cbb][:, mc * CBLK + sgo * 128:
                                    mc * CBLK + (sgo + 1) * 128],
                        tp[:])

            # issue all gathers; transpose first c-block's 4 groups now,
            # the rest are interleaved into GEMM1 below so the tensor queue
            # stays available.
            for sg in range(C // 128):
                dgs.append(gather_sg(sg))
            for sg in range(4):
                transpose_sg(sg, dgs[sg])

            # ---------------- phases 4+5: expert FFN ----------------
            ps_ffn_cm = tc.tile_pool(name="ps_ffn", bufs=1, space="PSUM")
            ps_ffn = ps_ffn_cm.__enter__()
            hTs = [bigp.tile([128, (H // 128) * CBLK], f16, name=f"hT{c}")
                   for c in range(NCB)]
            w1v = w1_sb[:].rearrange("p (mc h) -> p mc h", h=H)
            for cb in range(NCB):
                hT = hTs[cb]
                # GEMM1: hT[h, c] = relu(w1.T @ dispT + b1)
                for htb in range(H // CBLK):              # 8 blocks of 4 ht
                    for hi in range(CBLK // 128):         # 4 ht per block
                        ht_i = htb * (CBLK // 128) + hi
                        ps1 = ps_ffn.tile([128, CBLK], f32, tag="g1", bufs=4)
                        for mc in range(M // 128):
                            nc.tensor.matmul(
                                ps1[:],
                                w1v[:, mc,
                                    htb * CBLK + hi * 128:
                                    htb * CBLK + (hi + 1) * 128],
                                dispTs[cb][:, mc * CBLK:(mc + 1) * CBLK],
                                start=(mc == 0), stop=(mc == M // 128 - 1))
                        nc.scalar.activation(
                            hT[:, ht_i * CBLK:(ht_i + 1) * CBLK], ps1[:],
                            ACTF.Relu, bias=b1c[:, ht_i:ht_i + 1], scale=1.0)
                    if cb == 0 and htb < 4:
                        # second c-block's transposes, interleaved
                        transpose_sg(4 + htb, dgs[4 + htb])

                # gate row broadcast for this c-block
                g_row = wk.tile([1, CBLK], f32, tag="grow")
                nc.sync.dma_start(g_row[0:1, :],
                                  G_f[cb * CBLK:(cb + 1) * CBLK])
                gb_ps = ps_ffn.tile([128, CBLK], f32, tag="g1", bufs=4)
                nc.tensor.matmul(gb_ps[:], ones_row[:], g_row[:],
                                 start=True, stop=True)
                g_bc = wk.tile([128, CBLK], f32, tag="gbc")
                nc.vector.tensor_copy(g_bc[:], gb_ps[:])

                # GEMM2: eoT[m, c] = w2.T @ hT ; then (+b2) * gate
                for mt in range(M // 128):
                    w2ts = wk.tile([128, H], f16, tag="w2ts", bufs=2)
                    nc.scalar.dma_start(
                        w2ts[:], w2t_ap[mt].rearrange("p (hc m) -> p hc m", m=128))
                    ps2 = ps_ffn.tile([128, CBLK], f32, tag="g2", bufs=2)
                    for hc in range(H // 128):
                        nc.tensor.matmul(
                            ps2[:], w2ts[:, hc * 128:(hc + 1) * 128],
                            hT[:, hc * CBLK:(hc + 1) * CBLK],
                            start=(hc == 0), stop=(hc == H // 128 - 1))
                    eo_sb = wk.tile([128, CBLK], f16, tag="eo")
                    nc.vector.tensor_scalar(out=eo_sb[:], in0=ps2[:],
                                            scalar1=b2c[:, mt:mt + 1],
                                            scalar2=None, op0=OP.add)
                    nc.vector.tensor_mul(eo_sb[:], eo_sb[:], g_bc[:])
                    nc.sync.dma_start(
                        eot_ap[mt * 128:(mt + 1) * 128,
                               cb * CBLK:(cb + 1) * CBLK], eo_sb[:])

            ps_ffn_cm.__exit__(None, None, None)
            ps_tr_cm.__exit__(None, None, None)
            wk_cm.__exit__(None, None, None)
            bigp_cm.__exit__(None, None, None)

    nc.compile()
    _split_multi_waits(nc)
    return nc


_NC_CACHE = None


def _get_nc():
    global _NC_CACHE
    if _NC_CACHE is None:
        _NC_CACHE = build()
    return _NC_CACHE


def _make_in_maps(x, wg, w1, b1, w2, b2):
    x2 = np.ascontiguousarray(np.asarray(x, np.float32).reshape(S, M))
    x16 = np.ascontiguousarray(x2.astype(np.float16))
    wg = np.ascontiguousarray(np.asarray(wg, np.float32))
    w1 = np.asarray(w1, np.float32)
    b1 = np.asarray(b1, np.float32)
    w2 = np.asarray(w2, np.float32)
    b2 = np.asarray(b2, np.float32)
    in_maps = []
    for k in range(N_CORES):
        shard = x2[k * (S // N_CORES):(k + 1) * (S // N_CORES)]
        xts = np.ascontiguousarray(shard.T)                    # [M, S/8]
        w1k = np.ascontiguousarray(w1[k]).astype(np.float16)   # [M, H]
        b1ck = np.ascontiguousarray(b1[k].reshape(H // 128, 128).T)
        w2k = w2[k]                                            # [H, M]
        w2t = np.ascontiguousarray(
            w2k.reshape(H // 128, 128, M // 128, 128).transpose(2, 1, 0, 3)
        ).astype(np.float16)
        b2ck = np.ascontiguousarray(b2[k].reshape(M // 128, 128).T)
        eid = np.full((128, 1), k, np.float32)
        in_maps.append({
            "x16": x16, "xts": xts, "wg": wg, "w1": w1k, "b1c": b1ck,
            "w2t": w2t, "b2c": b2ck, "eid": eid,
        })
    return in_maps


def run_cores(x, wg, w1, b1, w2, b2, trace=False, tmpdir=None):
    nc = _get_nc()
    in_maps = _make_in_maps(x, wg, w1, b1, w2, b2)
    return run_bass_kernel_spmd(nc, in_maps, list(range(N_CORES)), trace=trace,
                                tmpdir=tmpdir)


def combine(results):
    out = np.zeros((S, M), np.float32)
    for k in range(N_CORES):
        r = results[k]
        eo = np.ascontiguousarray(r["EOT"].astype(np.float32).T)  # [C, M]
        A = r["A"][:, 0].astype(np.int64)
        G = r["G"][:, 0]
        valid = G > 0
        out[A[valid]] = eo[valid]
    return out.reshape(B, SQ, M)


def kernel(x, wg, w1, b1, w2, b2):
    res = run_cores(x, wg, w1, b1, w2, b2, trace=False)
    return combine(res.results)
